# revision 1
# baseline (speedup 1.0000x reference)
"""ChebConv GNN (3 layers, K=4) on 8 Trainium2 NeuronCores.

Sharding: nodes are partitioned across the 8 cores (graph parallel). A
load-balancing permutation (LPT on in-degree) relabels nodes so every core
owns NW windows of 128 dst nodes with near-equal edge counts. Each SpMM
(lhat application) gathers source-node feature rows from a replicated
node-major table in HBM via dma_gather, segment-sums them per 128-dst
window with a one-hot matmul on the TensorEngine, and the per-core slices
are re-replicated with an AllGather between Chebyshev hops.

Compute layout is feature-major ([feature, node] in SBUF) so the dense
W-matmuls need no transposes; node-major copies for the gather tables are
produced with PE transposes on the way out.
"""

import numpy as np

# ---------------- problem constants (hardcoded per contract) ----------------
N, E = 50000, 800000
F, HID, CLS, K = 128, 128, 40, 4
P = 128
CORES = 8
NW = 50                 # dst windows per core (must be even)
SL = NW * P             # 6400 nodes per core
NPAD = CORES * SL       # 51200 padded node count
HALF = NPAD // 2        # 25600 rows per half-table (int16-indexable)


# ---------------- host preprocessing ----------------
def _lpt_windows(indeg, n_windows, cap):
    """Assign nodes to windows (cap nodes each), balancing in-degree sums.
    Returns perm: old node id -> new node id."""
    import heapq
    order = np.argsort(-indeg, kind="stable")
    heap = [(0, wi) for wi in range(n_windows)]
    heapq.heapify(heap)
    counts = np.zeros(n_windows, np.int64)
    perm = np.empty(len(indeg), np.int64)
    for old in order:
        while True:
            load, wi = heapq.heappop(heap)
            if counts[wi] < cap:
                break
        perm[old] = wi * cap + counts[wi]
        counts[wi] += 1
        if counts[wi] < cap:
            heapq.heappush(heap, (load + int(indeg[old]), wi))
    return perm


def _preprocess(edge_src, edge_dst, n, cfg):
    """Compute norm weights, node permutation, and per-core padded edge data."""
    cores, nw, p = cfg["CORES"], cfg["NW"], P
    sl = nw * p
    npad = cores * sl
    half = npad // 2

    es = np.asarray(edge_src, np.int64)
    ed = np.asarray(edge_dst, np.int64)
    deg = np.bincount(es, minlength=n).astype(np.float32)
    dinv = np.where(deg > 0, 1.0 / np.sqrt(np.maximum(deg, 1.0)), 0.0).astype(
        np.float32
    )
    wnorm = (-dinv[es] * dinv[ed]).astype(np.float32)

    indeg = np.bincount(ed, minlength=n)
    perm = _lpt_windows(indeg, cores * nw, p)  # old -> new

    nsrc = perm[es]
    ndst = perm[ed]
    core_e = ndst // sl
    win_e = (ndst % sl) // p
    dloc_e = (ndst % p).astype(np.float32)
    half_e = (nsrc >= half).astype(np.int64)
    idx_e = (nsrc - half_e * half).astype(np.int64)

    # group edges by (core, win, half)
    gkey = (core_e * nw + win_e) * 2 + half_e
    ngroups = cores * nw * 2
    order = np.argsort(gkey, kind="stable")
    gkey_s = gkey[order]
    counts = np.bincount(gkey_s, minlength=ngroups)
    starts = np.concatenate([[0], np.cumsum(counts)[:-1]])
    rank = np.arange(len(es)) - starts[gkey_s]  # position within group

    cnts = counts.reshape(cores, nw, 2)
    CA = int(np.ceil(cnts[:, :, 0].max() / p))
    CB = int(np.ceil(cnts[:, :, 1].max() / p))
    CA = max(CA, 1)
    CB = max(CB, 1)
    CW = CA + CB

    # padded edge slot arrays
    capa = {0: CA * p, 1: CB * p}
    idx_pad = {h: np.zeros((cores, nw, capa[h]), np.int16) for h in (0, 1)}
    dl_pad = np.zeros((cores, nw, CW, p), np.float32)
    w_pad = np.zeros((cores, nw, CW, p), np.float32)

    ce, we, he = core_e[order], win_e[order], half_e[order]
    de, wne, ie = dloc_e[order], wnorm[order], idx_e[order]
    for h in (0, 1):
        m = he == h
        idx_pad[h][ce[m], we[m], rank[m]] = ie[m].astype(np.int16)
        coff = rank[m] // p + (0 if h == 0 else CA)
        dl_pad[ce[m], we[m], coff, rank[m] % p] = de[m]
        w_pad[ce[m], we[m], coff, rank[m] % p] = wne[m]

    # dma_gather index arrays per pair of windows: [cores, nw//2, 128, len/16]
    def wrap(idxs):  # idxs: [cores, nw//2, L] -> [cores, nw//2, 128, L//16]
        c, g, L = idxs.shape
        a = idxs.reshape(c, g, L // 16, 16).transpose(0, 1, 3, 2)  # [c,g,16,L/16]
        return np.tile(a, (1, 1, 8, 1)).copy()  # [c,g,128,L/16]

    idxA = wrap(idx_pad[0].reshape(cores, nw // 2, 2 * CA * p))
    idxB = wrap(idx_pad[1].reshape(cores, nw // 2, 2 * CB * p))

    # dl/w arrays in SBUF layout [cores, 128(p), nw*CW]
    dl_arr = dl_pad.transpose(0, 3, 1, 2).reshape(cores, p, nw * CW).copy()
    w_arr = w_pad.transpose(0, 3, 1, 2).reshape(cores, p, nw * CW).copy()

    return dict(
        perm=perm, wnorm=wnorm, CA=CA, CB=CB, CW=CW,
        idxA=idxA, idxB=idxB, dl=dl_arr, w=w_arr, w2=(2.0 * w_arr),
    )


# ---------------- device kernel ----------------
def _build(cfg, CA, CB):
    import concourse.bass as bass
    import concourse.bacc as bacc
    import concourse.tile as tile
    import concourse.mybir as mybir
    import dataclasses

    cores, nw = cfg["CORES"], cfg["NW"]
    sl = nw * P
    npad = cores * sl
    half = npad // 2
    CW = CA + CB
    fp = mybir.dt.float32
    bf = mybir.dt.bfloat16
    Alu = mybir.AluOpType
    Act = mybir.ActivationFunctionType

    nc = bacc.Bacc("TRN2", target_bir_lowering=False, debug=False,
                   num_devices=cores, num_swdge_queues=4)

    # -------- I/O --------
    xT_d = nc.dram_tensor("xT", [P, sl], fp, kind="ExternalInput")
    xfull_d = nc.dram_tensor("xfull", [npad, F], bf, kind="ExternalInput")
    idxA_d = nc.dram_tensor("idxA", [nw // 2, P, CA * 16], mybir.dt.int16,
                            kind="ExternalInput")
    idxB_d = nc.dram_tensor("idxB", [nw // 2, P, CB * 16], mybir.dt.int16,
                            kind="ExternalInput")
    dl_d = nc.dram_tensor("dl", [P, nw * CW], bf, kind="ExternalInput")
    wt_d = nc.dram_tensor("wt", [P, nw * CW], bf, kind="ExternalInput")
    wt2_d = nc.dram_tensor("wt2", [P, nw * CW], bf, kind="ExternalInput")
    w0_d = nc.dram_tensor("w0t", [P, K, HID], fp, kind="ExternalInput")
    w1_d = nc.dram_tensor("w1t", [P, K, HID], fp, kind="ExternalInput")
    w2_d = nc.dram_tensor("w2t", [P, K, CLS], fp, kind="ExternalInput")
    b0_d = nc.dram_tensor("b0", [HID, 1], fp, kind="ExternalInput")
    b1_d = nc.dram_tensor("b1", [HID, 1], fp, kind="ExternalInput")
    b2_d = nc.dram_tensor("b2", [CLS, 1], fp, kind="ExternalInput")
    iota_d = nc.dram_tensor("iota", [P, P], bf, kind="ExternalInput")
    ident_d = nc.dram_tensor("ident", [P, P], fp, kind="ExternalInput")
    out_d = nc.dram_tensor("out", [sl, CLS], fp, kind="ExternalOutput")

    def bcol(t, c):  # [128,1] column slice
        return t[:, c:c + 1]

    def bmid(ap, n):  # [128, X] -> [128, n, X], middle stride 0
        return dataclasses.replace(ap, ap=[ap.ap[0], [0, n], ap.ap[1]])

    def blast(ap, n):  # [128, X] -> [128, X, n], last stride 0
        return dataclasses.replace(ap, ap=[ap.ap[0], ap.ap[1], [0, n]])

    with tile.TileContext(nc) as tc:
        with (
            tc.tile_pool(name="const", bufs=1) as constp,
            tc.tile_pool(name="tx", bufs=3) as txp,
            tc.tile_pool(name="acc", bufs=1) as accp,
            tc.tile_pool(name="g", bufs=2) as gp,
            tc.tile_pool(name="m", bufs=2) as mp,
            tc.tile_pool(name="ix", bufs=2) as ixp,
            tc.tile_pool(name="st", bufs=4) as stp,
            tc.tile_pool(name="psA", bufs=2, space="PSUM") as psA,
            tc.tile_pool(name="psT", bufs=2, space="PSUM") as psT,
            tc.tile_pool(name="psW", bufs=2, space="PSUM") as psW,
            tc.tile_pool(name="dram", bufs=2, space="DRAM") as dramp,
            tc.tile_pool(name="tabs", bufs=3, space="DRAM") as tabp,
        ):
            # -------- constants --------
            dl_t = constp.tile([P, nw * CW], bf)
            wt_t = constp.tile([P, nw * CW], bf)
            wt2_t = constp.tile([P, nw * CW], bf)
            iota_t = constp.tile([P, P], bf)
            ident_t = constp.tile([P, P], fp)
            w0_t = constp.tile([P, K, HID], fp)
            w1_t = constp.tile([P, K, HID], fp)
            w2_t = constp.tile([P, K, CLS], fp)
            b0_t = constp.tile([HID, 1], fp)
            b1_t = constp.tile([HID, 1], fp)
            b2_t = constp.tile([CLS, 1], fp)
            for t, d in ((dl_t, dl_d), (wt_t, wt_d), (wt2_t, wt2_d),
                         (iota_t, iota_d), (ident_t, ident_d),
                         (w0_t, w0_d), (w1_t, w1_d), (w2_t, w2_d),
                         (b0_t, b0_d), (b1_t, b1_d), (b2_t, b2_d)):
                nc.sync.dma_start(out=t[:], in_=d[:])

            tx0 = txp.tile([P, sl], fp, tag="tx")
            nc.sync.dma_start(out=tx0[:], in_=xT_d[:, :])

            tabA_in = xfull_d[0:half, :]
            tabB_in = xfull_d[half:npad, :]

            def spmm(wsel_t, tabA, tabB, tx_prev2, Wt, fo, acc, k, want_slice):
                """One lhat application; returns (tx_new, slice_dram|None)."""
                tx_new = txp.tile([P, sl], fp, tag="tx")
                slice_d = (dramp.tile([sl, F], bf, tag="slice", name="slice_d")
                           if want_slice else None)
                nA, nB = 2 * CA * P, 2 * CB * P
                for g in range(nw // 2):
                    ixA = ixp.tile([P, CA * 16], mybir.dt.int16, tag="ixA")
                    nc.sync.dma_start(out=ixA[:], in_=idxA_d[g])
                    ixB = ixp.tile([P, CB * 16], mybir.dt.int16, tag="ixB")
                    nc.sync.dma_start(out=ixB[:], in_=idxB_d[g])
                    GA = gp.tile([P, 2 * CA, P], bf, tag="GA")
                    nc.gpsimd.dma_gather(
                        out_ap=GA[:], in_ap=tabA, idxs_ap=ixA[:],
                        num_idxs=nA, num_idxs_reg=nA, elem_size=P,
                        single_packet=False, queue_num=(2 * g) % 4)
                    GB = gp.tile([P, 2 * CB, P], bf, tag="GB")
                    nc.gpsimd.dma_gather(
                        out_ap=GB[:], in_ap=tabB, idxs_ap=ixB[:],
                        num_idxs=nB, num_idxs_reg=nB, elem_size=P,
                        single_packet=False, queue_num=(2 * g + 1) % 4)
                    for h in (0, 1):
                        w = 2 * g + h
                        wb = slice(w * P, (w + 1) * P)
                        colsl = slice(w * CW, (w + 1) * CW)
                        M = mp.tile([P, CW, P], bf, tag="M")
                        nc.vector.tensor_tensor(
                            out=M[:], in0=bmid(iota_t[:], CW),
                            in1=blast(dl_t[:, colsl], P), op=Alu.is_equal)
                        nc.vector.tensor_tensor(
                            out=M[:], in0=M[:],
                            in1=blast(wsel_t[:, colsl], P), op=Alu.mult)
                        ps = psA.tile([P, P], fp, tag="ps")
                        for c in range(CW):
                            Gsl = (GA[:, h * CA + c, :] if c < CA
                                   else GB[:, h * CB + (c - CA), :])
                            nc.tensor.matmul(out=ps[:], lhsT=Gsl, rhs=M[:, c, :],
                                             start=(c == 0), stop=(c == CW - 1))
                        if tx_prev2 is None:
                            nc.vector.tensor_copy(out=tx_new[:, wb], in_=ps[:])
                        else:
                            nc.vector.tensor_tensor(
                                out=tx_new[:, wb], in0=ps[:],
                                in1=tx_prev2[:, wb], op=Alu.subtract)
                        psw = psW.tile([P, P], fp, tag="psw")
                        nc.tensor.matmul(out=psw[:fo, :], lhsT=Wt[:, k, :fo],
                                         rhs=tx_new[:, wb], start=True, stop=True)
                        nc.vector.tensor_tensor(out=acc[:fo, wb], in0=acc[:fo, wb],
                                                in1=psw[:fo, :], op=Alu.add)
                        if slice_d is not None:
                            pst = psT.tile([P, P], fp, tag="pst")
                            nc.tensor.transpose(out=pst[:], in_=tx_new[:, wb],
                                                identity=ident_t[:])
                            st = stp.tile([P, P], bf, tag="st")
                            nc.scalar.copy(out=st[:], in_=pst[:])
                            nc.scalar.dma_start(out=slice_d[w * P:(w + 1) * P, :],
                                                in_=st[:])
                return tx_new, slice_d

            def allgather(slice_d):
                tab = tabp.tile([npad, F], bf, tag="tab", addr_space="Shared")
                nc.gpsimd.collective_compute(
                    "AllGather", Alu.bypass,
                    replica_groups=[list(range(cores))],
                    ins=[slice_d[:, :].opt()], outs=[tab[:, :].opt()])
                return tab

            stage = cfg.get("STAGE", 99)
            for l, (Wt, b_t, fo) in enumerate(
                    ((w0_t, b0_t, HID), (w1_t, b1_t, HID), (w2_t, b2_t, CLS))):
                if l * 10 >= stage:
                    break
                last = l == 2
                acc = accp.tile([P, sl], fp, tag="acc")
                # ---- k=0 term: acc = W[0].T @ tx0 + b ----
                for w in range(nw):
                    wb = slice(w * P, (w + 1) * P)
                    psw = psW.tile([P, P], fp, tag="psw")
                    nc.tensor.matmul(out=psw[:fo, :], lhsT=Wt[:, 0, :fo],
                                     rhs=tx0[:, wb], start=True, stop=True)
                    nc.vector.tensor_scalar(
                        out=acc[:fo, wb], in0=psw[:fo, :],
                        scalar1=b_t[:fo, 0:1], scalar2=None, op0=Alu.add)
                # ---- k=1..3 ----
                if stage < l * 10 + 2:
                    break
                tx1, sl1 = spmm(wt_t, tabA_in, tabB_in, None, Wt, fo, acc, 1,
                                stage >= l * 10 + 3)
                if stage < l * 10 + 3:
                    break
                t1 = allgather(sl1)
                if stage < l * 10 + 4:
                    break
                tx2, sl2 = spmm(wt2_t, t1[0:half, :], t1[half:npad, :], tx0,
                                Wt, fo, acc, 2, stage >= l * 10 + 5)
                if stage < l * 10 + 5:
                    break
                t2 = allgather(sl2)
                if stage < l * 10 + 6:
                    break
                tx3, _ = spmm(wt2_t, t2[0:half, :], t2[half:npad, :], tx1,
                              Wt, fo, acc, 3, False)
                if stage < l * 10 + 7:
                    break
                # ---- epilogue ----
                if not last:
                    hT = txp.tile([P, sl], fp, tag="tx")
                    slice_h = dramp.tile([sl, F], bf, tag="slice")
                    for w in range(nw):
                        wb = slice(w * P, (w + 1) * P)
                        nc.scalar.activation(out=hT[:, wb], in_=acc[:, wb],
                                             func=Act.Relu)
                        pst = psT.tile([P, P], fp, tag="pst")
                        nc.tensor.transpose(out=pst[:], in_=hT[:, wb],
                                            identity=ident_t[:])
                        st = stp.tile([P, P], bf, tag="st")
                        nc.scalar.copy(out=st[:], in_=pst[:])
                        nc.scalar.dma_start(out=slice_h[w * P:(w + 1) * P, :],
                                            in_=st[:])
                    th = allgather(slice_h)
                    tx0 = hT
                    tabA_in, tabB_in = th[0:half, :], th[half:npad, :]
                else:
                    for w in range(nw):
                        wb = slice(w * P, (w + 1) * P)
                        pst = psT.tile([P, P], fp, tag="pst")
                        nc.tensor.transpose(out=pst[:, :CLS], in_=acc[:CLS, wb],
                                            identity=ident_t[:CLS, :CLS])
                        nm = stp.tile([P, 1], fp, tag="nm")
                        nc.vector.tensor_reduce(
                            out=nm[:], in_=pst[:, :CLS], op=Alu.max,
                            axis=mybir.AxisListType.X, negate=True)
                        ex = stp.tile([P, CLS], fp, tag="ex")
                        ssum = stp.tile([P, 1], fp, tag="ssum")
                        nc.scalar.activation(out=ex[:], in_=pst[:, :CLS],
                                             func=Act.Exp, bias=nm[:, 0:1],
                                             accum_out=ssum[:, 0:1])
                        lse = stp.tile([P, 1], fp, tag="lse")
                        nc.scalar.activation(out=lse[:], in_=ssum[:], func=Act.Ln)
                        res = stp.tile([P, CLS], fp, tag="res")
                        nc.vector.tensor_scalar(
                            out=res[:], in0=pst[:, :CLS],
                            scalar1=nm[:, 0:1], scalar2=lse[:, 0:1],
                            op0=Alu.add, op1=Alu.subtract)
                        nc.scalar.dma_start(out=out_d[w * P:(w + 1) * P, :],
                                            in_=res[:])

    nc.compile()
    return nc


_CACHE = {}


def _get_nc(cfg, CA, CB):
    key = (cfg["CORES"], cfg["NW"], CA, CB, cfg.get("STAGE", 99))
    if key not in _CACHE:
        _CACHE[key] = _build(cfg, CA, CB)
    return _CACHE[key]


def _run(x, edge_src, edge_dst, W0, b0, W1, b1, W2, b2, cfg=None,
         trace=False, trace_cores=None):
    from concourse import bass_utils

    cfg = cfg or {"CORES": CORES, "NW": NW}
    cores, nw = cfg["CORES"], cfg["NW"]
    sl = nw * P
    npad = cores * sl
    n = x.shape[0]

    import ml_dtypes
    bf16 = ml_dtypes.bfloat16

    pre = _preprocess(edge_src, edge_dst, n, cfg)
    perm, CA, CB = pre["perm"], pre["CA"], pre["CB"]

    x = np.asarray(x, np.float32)
    x_pad = np.zeros((npad, F), np.float32)
    x_pad[perm] = x

    w0t = np.ascontiguousarray(np.transpose(np.asarray(W0, np.float32), (1, 0, 2)))
    w1t = np.ascontiguousarray(np.transpose(np.asarray(W1, np.float32), (1, 0, 2)))
    w2t = np.ascontiguousarray(np.transpose(np.asarray(W2, np.float32), (1, 0, 2)))
    iota = np.broadcast_to(np.arange(P, dtype=np.float32), (P, P)).copy()
    ident = np.eye(P, dtype=np.float32)

    in_maps = []
    for c in range(cores):
        rows = slice(c * sl, (c + 1) * sl)
        in_maps.append(dict(
            xT=np.ascontiguousarray(x_pad[rows].T),
            xfull=x_pad.astype(bf16),
            idxA=pre["idxA"][c], idxB=pre["idxB"][c],
            dl=pre["dl"][c].astype(bf16), wt=pre["w"][c].astype(bf16),
            wt2=pre["w2"][c].astype(bf16),
            w0t=w0t, w1t=w1t, w2t=w2t,
            b0=np.asarray(b0, np.float32).reshape(HID, 1),
            b1=np.asarray(b1, np.float32).reshape(HID, 1),
            b2=np.asarray(b2, np.float32).reshape(CLS, 1),
            iota=iota.astype(bf16), ident=ident,
        ))

    nc = _get_nc(cfg, CA, CB)
    kw = {}
    if trace:
        kw = dict(trace=True,
                  trace_cores=trace_cores if trace_cores is not None else [0])
    res = bass_utils.run_bass_kernel_spmd(nc, in_maps,
                                          core_ids=list(range(cores)), **kw)

    full = np.concatenate([res.results[c]["out"] for c in range(cores)], axis=0)
    out = full[perm]  # inverse permutation: row for old node i is at full[perm[i]]
    return out.astype(np.float32), res


def kernel(x, edge_src, edge_dst, W0, b0, W1, b1, W2, b2):
    out, _ = _run(x, edge_src, edge_dst, W0, b0, W1, b1, W2, b2)
    return out



# revision 4
# speedup vs baseline: 1.3321x; 1.3321x over previous
"""ChebConv GNN (3 layers, K=4) on 8 Trainium2 NeuronCores.

Node-parallel sharding: an LPT permutation (on in-degree) relabels nodes into
400 windows of 128 dst nodes (50 windows per core). Each SpMM hop gathers
source rows from a replicated node-major HBM table (dma_gather, one 256B
descriptor per edge) and segment-sums them per window with one-hot matmuls on
the TensorEngine.

Key structure vs. a naive implementation:
 - The edge weight -dinv[src]*dinv[dst] is separable: dinv[src] is folded
   into the gather tables (applied when slices are produced), -2*dinv[dst]
   is applied to the segment-sum output per window. The one-hot matrices are
   then static 0/1, identical for all 9 SpMMs: they are host-built in fp8
   and cached in SBUF (matmul takes bf16 lhsT x fp8 rhs), with the overflow
   built on the fly by DVE from compressed dst-slot data.
 - The inter-hop AllGather is split in two (windows 0..29 / 30..49) so the
   first half fires mid-hop; next hop's gathers from the A-half table start
   while the B-half collective is still in flight. A-source gathers lead
   B-source gathers by a few pairs to cover the tail.
 - Everything flows in bf16 (fp32 PSUM accumulation), weights included.
"""

import numpy as np

# ---------------- problem constants (hardcoded per contract) ----------------
N, E = 50000, 800000
F, HID, CLS, K = 128, 128, 40, 4
P = 128
CORES = 8
NW = 50                  # dst windows per core
SL = NW * P              # 6400 nodes per core
NPAD = CORES * SL        # 51200 padded node count
WA = 30                  # windows in the A half (0..29)
WB = NW - WA             # windows in the B half (30..49)
NA = WA * P              # 3840 A-nodes per core
NB = WB * P              # 2560 B-nodes per core
RA = CORES * NA          # 30720 rows in table A (int16-indexable)
RB = CORES * NB          # 20480 rows in table B

LEAD = 3                 # A-gather lead (pairs) over B-gathers
NCHC_MAX = 460           # max one-hot chunks cached in SBUF (fp8)


# ---------------- host preprocessing ----------------
def _lpt_windows(indeg, n_windows, cap):
    """Assign nodes to windows (cap nodes each), balancing in-degree sums.
    Returns perm: old node id -> new node id."""
    import heapq
    order = np.argsort(-indeg, kind="stable")
    heap = [(0, wi) for wi in range(n_windows)]
    heapq.heapify(heap)
    counts = np.zeros(n_windows, np.int64)
    perm = np.empty(len(indeg), np.int64)
    for old in order:
        while True:
            load, wi = heapq.heappop(heap)
            if counts[wi] < cap:
                break
        perm[old] = wi * cap + counts[wi]
        counts[wi] += 1
        if counts[wi] < cap:
            heapq.heappush(heap, (load + int(indeg[old]), wi))
    return perm


def _wrap_idx(flat):
    """Logical idx list [L] -> [128, L//16] wrapped layout for dma_gather."""
    L = flat.shape[0]
    a = flat.reshape(L // 16, 16).T            # [16, L/16]
    return np.tile(a, (8, 1))                  # [128, L/16]


def _preprocess(edge_src, edge_dst, n):
    import ml_dtypes
    bf16 = ml_dtypes.bfloat16
    fp8 = ml_dtypes.float8_e4m3fn

    es = np.asarray(edge_src, np.int64)
    ed = np.asarray(edge_dst, np.int64)
    deg = np.bincount(es, minlength=n).astype(np.float32)
    dinv = np.where(deg > 0, 1.0 / np.sqrt(np.maximum(deg, 1.0)), 0.0).astype(
        np.float32
    )
    indeg = np.bincount(ed, minlength=n)
    perm = _lpt_windows(indeg, CORES * NW, P)      # old -> new
    dinv_new = np.zeros(NPAD, np.float32)
    dinv_new[perm] = dinv

    nsrc = perm[es]
    ndst = perm[ed]
    # source table rows
    score = nsrc // SL
    sloc = nsrc % SL
    half_e = (sloc >= NA).astype(np.int64)         # 0 = A, 1 = B
    srow = np.where(half_e == 0, score * NA + sloc, score * NB + (sloc - NA))
    # dst decomposition
    dcore = ndst // SL
    dl6400 = ndst % SL
    dwin = dl6400 // P
    dloc = dl6400 % P

    # per (core, window, half) counts -> global per-(window, half) chunk caps
    key = (dcore * NW + dwin) * 2 + half_e
    cnt = np.bincount(key, minlength=CORES * NW * 2).reshape(CORES, NW, 2)
    C = np.maximum(1, (cnt.max(axis=0) + P - 1) // P)   # [NW, 2] chunks
    moff = np.zeros((NW, 2), np.int64)                  # chunk-id offsets
    run = 0
    for w in range(NW):
        for h in (0, 1):
            moff[w, h] = run
            run += C[w, h]
    NCH = int(run)

    # slot assignment: rank within (core, window, half)
    order = np.argsort(key, kind="stable")
    key_s = key[order]
    counts_flat = np.bincount(key_s, minlength=CORES * NW * 2)
    starts = np.concatenate([[0], np.cumsum(counts_flat)[:-1]])
    rank = np.arange(len(es)) - starts[key_s]

    ce, we, he = dcore[order], dwin[order], half_e[order]
    de, se, re_ = dloc[order], srow[order], rank

    # build per-core idx arrays, M one-hots, dl (compressed dst-slot)
    idx_cols = int((C[:, 0].sum() + C[:, 1].sum()) * 8)   # NCH * 8
    idxs = np.zeros((CORES, P, idx_cols), np.int16)
    Mfull = np.zeros((CORES, P, NCH, P), np.uint8)
    dl = np.full((CORES, P, NCH), -1.0, np.float32)

    # idx column offsets per (pair, half): pair g covers windows 2g, 2g+1
    nA_pair = (C[0::2, 0] + C[1::2, 0]) * P               # [25]
    nB_pair = (C[0::2, 1] + C[1::2, 1]) * P
    iAoff = np.zeros(NW // 2, np.int64)
    iBoff = np.zeros(NW // 2, np.int64)
    off = 0
    for g in range(NW // 2):
        iAoff[g] = off
        off += nA_pair[g] // 16
        iBoff[g] = off
        off += nB_pair[g] // 16
    assert off == idx_cols

    # flat slot id per edge within its (window, half) block
    chunk_g = moff[we, he] + re_ // P      # global chunk id
    spart = re_ % P                        # slot partition
    Mfull[ce, spart, chunk_g, de] = 1
    dl[ce, spart, chunk_g] = de

    # idx flat arrays: for each (c, g, h): concat windows (2g, 2g+1) blocks,
    # each padded to C[w,h]*128 slots (pad idx 0)
    idx_flat = np.zeros((CORES, NCH * P), np.int64)
    slot_in_blk = moff[we, he] * P + re_
    idx_flat[ce, slot_in_blk] = se
    for c in range(CORES):
        for g in range(NW // 2):
            for h, ioff in ((0, iAoff[g]), (1, iBoff[g])):
                w0, w1 = 2 * g, 2 * g + 1
                blk = np.concatenate([
                    idx_flat[c, moff[w0, h] * P:(moff[w0, h] + C[w0, h]) * P],
                    idx_flat[c, moff[w1, h] * P:(moff[w1, h] + C[w1, h]) * P],
                ])
                wrapped = _wrap_idx(blk.astype(np.int16))
                idxs[c, :, ioff:ioff + blk.shape[0] // 16] = wrapped

    nchc = min(NCHC_MAX, NCH)
    mcache = np.ascontiguousarray(Mfull[:, :, :nchc, :]).astype(fp8)

    # per-core constants
    dinv_c = dinv_new.reshape(CORES, SL)
    dinvd2 = np.broadcast_to((-2.0 * dinv_c)[:, None, :], (CORES, P, SL))
    dinvn = dinv_c.reshape(CORES, NW, P).transpose(0, 2, 1)  # [c, 128, NW]

    return dict(
        perm=perm, dinv_new=dinv_new, C=C, moff=moff, NCH=NCH, nchc=nchc,
        nA_pair=nA_pair.astype(np.int64), nB_pair=nB_pair.astype(np.int64),
        iAoff=iAoff, iBoff=iBoff, idxs=idxs, mcache=mcache,
        dl=np.ascontiguousarray(dl.astype(bf16)),
        dinvd2=np.ascontiguousarray(dinvd2.astype(bf16)),
        dinvn=np.ascontiguousarray(dinvn.astype(np.float32)),
        idx_cols=idx_cols,
    )


# ---------------- device kernel ----------------
def _build(sched):
    import concourse.bass as bass
    import concourse.bacc as bacc
    import concourse.tile as tile
    import concourse.mybir as mybir
    import dataclasses

    C = np.asarray(sched["C"], np.int64).reshape(NW, 2)
    moff = np.asarray(sched["moff"], np.int64).reshape(NW, 2)
    NCH = int(sched["NCH"])
    NCHC = int(sched["nchc"])
    nA_pair = np.asarray(sched["nA_pair"], np.int64)
    nB_pair = np.asarray(sched["nB_pair"], np.int64)
    iAoff = np.asarray(sched["iAoff"], np.int64)
    iBoff = np.asarray(sched["iBoff"], np.int64)
    idx_cols = int(sched["idx_cols"])
    CAmax = int((C[0::2, 0] + C[1::2, 0]).max())
    CBmax = int((C[0::2, 1] + C[1::2, 1]).max())
    stage = int(sched.get("STAGE", 99))

    fp = mybir.dt.float32
    bf = mybir.dt.bfloat16
    f8 = mybir.dt.float8e4
    Alu = mybir.AluOpType
    Act = mybir.ActivationFunctionType

    nc = bacc.Bacc("TRN2", target_bir_lowering=False, debug=False,
                   num_devices=CORES, num_swdge_queues=4)

    # -------- I/O --------
    xT_d = nc.dram_tensor("xT", [P, SL], bf, kind="ExternalInput")
    xA_d = nc.dram_tensor("xA", [RA, F], bf, kind="ExternalInput")
    xB_d = nc.dram_tensor("xB", [RB, F], bf, kind="ExternalInput")
    idx_d = nc.dram_tensor("idxs", [P, idx_cols], mybir.dt.int16,
                           kind="ExternalInput")
    mc_d = nc.dram_tensor("mcache", [P, NCHC, P], f8, kind="ExternalInput")
    dl_d = nc.dram_tensor("dl", [P, NCH], bf, kind="ExternalInput")
    dinvd2_d = nc.dram_tensor("dinvd2", [P, SL], bf, kind="ExternalInput")
    dinvn_d = nc.dram_tensor("dinvn", [P, NW], fp, kind="ExternalInput")
    w0_d = nc.dram_tensor("w0t", [P, K, HID], bf, kind="ExternalInput")
    w1_d = nc.dram_tensor("w1t", [P, K, HID], bf, kind="ExternalInput")
    w2_d = nc.dram_tensor("w2t", [P, K, CLS], bf, kind="ExternalInput")
    b0_d = nc.dram_tensor("b0", [HID, 1], fp, kind="ExternalInput")
    b1_d = nc.dram_tensor("b1", [HID, 1], fp, kind="ExternalInput")
    b2_d = nc.dram_tensor("b2", [CLS, 1], fp, kind="ExternalInput")
    iota_d = nc.dram_tensor("iota", [P, P], bf, kind="ExternalInput")
    identb_d = nc.dram_tensor("identb", [P, P], bf, kind="ExternalInput")
    ident_d = nc.dram_tensor("ident", [P, P], fp, kind="ExternalInput")
    out_d = nc.dram_tensor("out", [SL, CLS], fp, kind="ExternalOutput")

    def bmid(ap, n):  # [128, X] -> [128, n, X], middle stride 0
        return dataclasses.replace(ap, ap=[ap.ap[0], [0, n], ap.ap[1]])

    def blast(ap, n):  # [128, X] -> [128, X, n], last stride 0
        return dataclasses.replace(ap, ap=[ap.ap[0], ap.ap[1], [0, n]])

    qctr = [0]

    def nxtq():
        qctr[0] = (qctr[0] + 1) % 4
        return qctr[0]

    with tile.TileContext(nc) as tc:
        with (
            tc.tile_pool(name="const", bufs=1) as constp,
            tc.tile_pool(name="tx", bufs=4) as txp,
            tc.tile_pool(name="acc", bufs=1) as accp,
            tc.tile_pool(name="gA", bufs=LEAD + 2) as gAp,
            tc.tile_pool(name="gB", bufs=2) as gBp,
            tc.tile_pool(name="tmp", bufs=2) as tmpp,
            tc.tile_pool(name="mb", bufs=2) as mbp,
            tc.tile_pool(name="st", bufs=4) as stp,
            tc.tile_pool(name="psA", bufs=2, space="PSUM") as psA,
            tc.tile_pool(name="psT", bufs=2, space="PSUM") as psT,
            tc.tile_pool(name="psW", bufs=2, space="PSUM") as psW,
            tc.tile_pool(name="slA", bufs=2, space="DRAM") as slAp,
            tc.tile_pool(name="slB", bufs=2, space="DRAM") as slBp,
            tc.tile_pool(name="tabA", bufs=2, space="DRAM") as tabAp,
            tc.tile_pool(name="tabB", bufs=2, space="DRAM") as tabBp,
        ):
            # -------- constants --------
            mc_t = constp.tile([P, NCHC, P], f8)
            idx_t = constp.tile([P, idx_cols], mybir.dt.int16)
            dl_t = constp.tile([P, NCH], bf)
            dinvd2_t = constp.tile([P, SL], bf)
            dinvn_t = constp.tile([P, NW], fp)
            iota_t = constp.tile([P, P], bf)
            identb_t = constp.tile([P, P], bf)
            ident_t = constp.tile([P, P], fp)
            w0_t = constp.tile([P, K, HID], bf)
            w1_t = constp.tile([P, K, HID], bf)
            w2_t = constp.tile([P, K, CLS], bf)
            b0_t = constp.tile([HID, 1], fp)
            b1_t = constp.tile([HID, 1], fp)
            b2_t = constp.tile([CLS, 1], fp)
            for t, d in ((mc_t, mc_d), (idx_t, idx_d), (dl_t, dl_d),
                         (dinvd2_t, dinvd2_d), (dinvn_t, dinvn_d),
                         (iota_t, iota_d), (identb_t, identb_d),
                         (ident_t, ident_d),
                         (w0_t, w0_d), (w1_t, w1_d), (w2_t, w2_d),
                         (b0_t, b0_d), (b1_t, b1_d), (b2_t, b2_d)):
                nc.sync.dma_start(out=t[:], in_=d[:])

            tx0 = txp.tile([P, SL], bf, tag="tx")
            nc.sync.dma_start(out=tx0[:], in_=xT_d[:, :])

            def seg_rhs(ms):
                """rhs AP for global one-hot chunk ms."""
                if ms < NCHC:
                    return mc_t[:, ms, :]
                mb = mbp.tile([P, 1, P], bf, tag="mb")
                nc.vector.tensor_tensor(
                    out=mb[:], in0=bmid(iota_t[:], 1),
                    in1=blast(dl_t[:, ms:ms + 1], P), op=Alu.is_equal)
                return mb[:, 0, :]

            def spmm(tabA_ap, tabB_ap, tx_prev2, Wt, fo, acc, k, hctx):
                """One lhat hop. hctx = (last_layer, hT, sliceA, sliceB,
                dinvn) context for the k==3 fused epilogue."""
                last, hT, slA_t, slB_t = hctx
                tx_new = txp.tile([P, SL], bf, tag="tx")
                mk_slice = k < 3
                ga = {}

                def issue_A(g):
                    t = gAp.tile([P, CAmax, P], bf, tag="GA")
                    ca = int((nA_pair[g]) // P)
                    nc.gpsimd.dma_gather(
                        out_ap=t[:, :ca, :], in_ap=tabA_ap,
                        idxs_ap=idx_t[:, iAoff[g]:iAoff[g] + nA_pair[g] // 16],
                        num_idxs=int(nA_pair[g]), num_idxs_reg=int(nA_pair[g]),
                        elem_size=P, single_packet=False, queue_num=nxtq())
                    ga[g] = t

                for g in range(min(LEAD, NW // 2)):
                    issue_A(g)

                for g in range(NW // 2):
                    if g + LEAD < NW // 2:
                        issue_A(g + LEAD)
                    gb = gBp.tile([P, CBmax, P], bf, tag="GB")
                    cb = int(nB_pair[g] // P)
                    nc.gpsimd.dma_gather(
                        out_ap=gb[:, :cb, :], in_ap=tabB_ap,
                        idxs_ap=idx_t[:, iBoff[g]:iBoff[g] + nB_pair[g] // 16],
                        num_idxs=int(nB_pair[g]), num_idxs_reg=int(nB_pair[g]),
                        elem_size=P, single_packet=False, queue_num=nxtq())
                    gat = ga.pop(g)
                    aoff = 0 if True else 0
                    for wloc in (0, 1):
                        w = 2 * g + wloc
                        wb = slice(w * P, (w + 1) * P)
                        ps = psA.tile([P, P], fp, tag="ps")
                        na0 = int(C[2 * g, 0])
                        nb0 = int(C[2 * g, 1])
                        chunks = []
                        if wloc == 0:
                            chunks += [(gat, i, int(moff[w, 0]) + i)
                                       for i in range(int(C[w, 0]))]
                            chunks += [(gb, i, int(moff[w, 1]) + i)
                                       for i in range(int(C[w, 1]))]
                        else:
                            chunks += [(gat, na0 + i, int(moff[w, 0]) + i)
                                       for i in range(int(C[w, 0]))]
                            chunks += [(gb, nb0 + i, int(moff[w, 1]) + i)
                                       for i in range(int(C[w, 1]))]
                        nchk = len(chunks)
                        for j, (buf, lc, ms) in enumerate(chunks):
                            nc.tensor.matmul(
                                out=ps[:], lhsT=buf[:, lc, :], rhs=seg_rhs(ms),
                                start=(j == 0), stop=(j == nchk - 1))
                        tmp = tmpp.tile([P, P], fp, tag="tmp")
                        nc.vector.tensor_tensor(
                            out=tmp[:], in0=ps[:], in1=dinvd2_t[:, wb],
                            op=Alu.mult)
                        if k == 1:
                            nc.vector.tensor_scalar(
                                out=tx_new[:, wb], in0=tmp[:], scalar1=0.5,
                                scalar2=None, op0=Alu.mult)
                        else:
                            nc.vector.tensor_tensor(
                                out=tx_new[:, wb], in0=tmp[:],
                                in1=tx_prev2[:, wb], op=Alu.subtract)
                        psw = psW.tile([P, P], fp, tag="psw")
                        nc.tensor.matmul(out=psw[:fo, :], lhsT=Wt[:, k, :fo],
                                         rhs=tx_new[:, wb], start=True,
                                         stop=True)
                        nc.vector.tensor_tensor(out=acc[:fo, wb],
                                                in0=acc[:fo, wb],
                                                in1=psw[:fo, :], op=Alu.add)
                        if mk_slice:
                            pst = psT.tile([P, P], bf, tag="pst")
                            nc.tensor.transpose(out=pst[:], in_=tx_new[:, wb],
                                                identity=identb_t[:])
                            st = stp.tile([P, P], bf, tag="st")
                            nc.scalar.activation(out=st[:], in_=pst[:],
                                                 func=Act.Copy,
                                                 scale=dinvn_t[:, w:w + 1])
                            if w < WA:
                                nc.scalar.dma_start(
                                    out=slA_t[w * P:(w + 1) * P, :], in_=st[:])
                            else:
                                nc.scalar.dma_start(
                                    out=slB_t[(w - WA) * P:(w - WA + 1) * P, :],
                                    in_=st[:])
                        elif not last:
                            # k == 3: finish acc, produce h slice + hT
                            nc.scalar.activation(out=hT[:, wb],
                                                 in_=acc[:, wb], func=Act.Relu)
                            pst = psT.tile([P, P], bf, tag="pst")
                            nc.tensor.transpose(out=pst[:], in_=hT[:, wb],
                                                identity=identb_t[:])
                            st = stp.tile([P, P], bf, tag="st")
                            nc.scalar.activation(out=st[:], in_=pst[:],
                                                 func=Act.Copy,
                                                 scale=dinvn_t[:, w:w + 1])
                            if w < WA:
                                nc.scalar.dma_start(
                                    out=slA_t[w * P:(w + 1) * P, :], in_=st[:])
                            else:
                                nc.scalar.dma_start(
                                    out=slB_t[(w - WA) * P:(w - WA + 1) * P, :],
                                    in_=st[:])
                        else:
                            # k == 3, last layer: log_softmax epilogue
                            pst = psT.tile([P, P], fp, tag="pst32")
                            nc.tensor.transpose(out=pst[:, :CLS],
                                                in_=acc[:CLS, wb],
                                                identity=ident_t[:CLS, :CLS])
                            nm = stp.tile([P, 1], fp, tag="nm")
                            nc.vector.tensor_reduce(
                                out=nm[:], in_=pst[:, :CLS], op=Alu.max,
                                axis=mybir.AxisListType.X, negate=True)
                            ex = stp.tile([P, CLS], fp, tag="ex")
                            ssum = stp.tile([P, 1], fp, tag="ssum")
                            nc.scalar.activation(out=ex[:], in_=pst[:, :CLS],
                                                 func=Act.Exp, bias=nm[:, 0:1],
                                                 accum_out=ssum[:, 0:1])
                            lse = stp.tile([P, 1], fp, tag="lse")
                            nc.scalar.activation(out=lse[:], in_=ssum[:],
                                                 func=Act.Ln)
                            res = stp.tile([P, CLS], fp, tag="res")
                            nc.vector.tensor_scalar(
                                out=res[:], in0=pst[:, :CLS],
                                scalar1=nm[:, 0:1], scalar2=lse[:, 0:1],
                                op0=Alu.add, op1=Alu.subtract)
                            nc.scalar.dma_start(out=out_d[w * P:(w + 1) * P, :],
                                                in_=res[:])
                    # fire the A-half collective once windows 0..WA-1 done
                    if g == WA // 2 - 1 and (mk_slice or not last):
                        tabA_new = tabAp.tile([RA, F], bf, tag="tabA",
                                              addr_space="Shared")
                        nc.gpsimd.collective_compute(
                            "AllGather", Alu.bypass,
                            replica_groups=[list(range(CORES))],
                            ins=[slA_t[:, :].opt()],
                            outs=[tabA_new[:, :].opt()])
                        hctx2 = tabA_new
                    elif g == WA // 2 - 1:
                        hctx2 = None
                if mk_slice or not last:
                    tabB_new = tabBp.tile([RB, F], bf, tag="tabB",
                                          addr_space="Shared")
                    nc.gpsimd.collective_compute(
                        "AllGather", Alu.bypass,
                        replica_groups=[list(range(CORES))],
                        ins=[slB_t[:, :].opt()],
                        outs=[tabB_new[:, :].opt()])
                    return tx_new, hctx2, tabB_new
                return tx_new, None, None

            tabA_cur = xA_d[0:RA, :]
            tabB_cur = xB_d[0:RB, :]
            for l, (Wt, b_t, fo) in enumerate(
                    ((w0_t, b0_t, HID), (w1_t, b1_t, HID), (w2_t, b2_t, CLS))):
                if l * 10 >= stage:
                    break
                last = l == 2
                acc = accp.tile([P, SL], fp, tag="acc")
                # ---- k=0 term ----
                for w in range(NW):
                    wb = slice(w * P, (w + 1) * P)
                    psw = psW.tile([P, P], fp, tag="psw")
                    nc.tensor.matmul(out=psw[:fo, :], lhsT=Wt[:, 0, :fo],
                                     rhs=tx0[:, wb], start=True, stop=True)
                    nc.vector.tensor_scalar(
                        out=acc[:fo, wb], in0=psw[:fo, :],
                        scalar1=b_t[:fo, 0:1], scalar2=None, op0=Alu.add)
                # ---- hops ----
                hT = (None if last
                      else txp.tile([P, SL], bf, tag="tx", name="hT"))
                tx1 = tx2 = tx3 = None
                for k in (1, 2, 3):
                    if stage < l * 10 + k + 1:
                        break
                    mk_slice = k < 3
                    slA_t = (slAp.tile([NA, F], bf, tag="slA", name="slA_t")
                             if (mk_slice or not last) else None)
                    slB_t = (slBp.tile([NB, F], bf, tag="slB", name="slB_t")
                             if (mk_slice or not last) else None)
                    prev2 = None if k == 1 else (tx0 if k == 2 else tx1)
                    txn, tA, tB = spmm(tabA_cur, tabB_cur, prev2, Wt, fo, acc,
                                       k, (last, hT, slA_t, slB_t))
                    if k == 1:
                        tx1 = txn
                    elif k == 2:
                        tx2 = txn
                    else:
                        tx3 = txn
                    if tA is not None:
                        tabA_cur = tA[0:RA, :]
                        tabB_cur = tB[0:RB, :]
                if not last:
                    tx0 = hT

    nc.compile()
    return nc


_CACHE = {}


def _get_nc(sched):
    key = (tuple(np.asarray(sched["C"]).flatten().tolist()),
           sched["NCH"], sched["nchc"], sched.get("STAGE", 99))
    if key not in _CACHE:
        _CACHE[key] = _build(sched)
    return _CACHE[key]


def _run(x, edge_src, edge_dst, W0, b0, W1, b1, W2, b2, cfg=None,
         trace=False, trace_cores=None):
    from concourse import bass_utils
    import ml_dtypes
    bf16 = ml_dtypes.bfloat16

    n = x.shape[0]
    pre = _preprocess(edge_src, edge_dst, n)
    if cfg and "STAGE" in cfg:
        pre["STAGE"] = cfg["STAGE"]
    perm = pre["perm"]

    x = np.asarray(x, np.float32)
    x_pad = np.zeros((NPAD, F), np.float32)
    x_pad[perm] = x
    xs = x_pad * pre["dinv_new"][:, None]          # dinv-scaled rows
    xs_c = xs.reshape(CORES, NW, P, F)
    xA = np.ascontiguousarray(
        xs_c[:, :WA].reshape(CORES * NA, F)).astype(bf16)
    xB = np.ascontiguousarray(
        xs_c[:, WA:].reshape(CORES * NB, F)).astype(bf16)

    w0t = np.ascontiguousarray(
        np.transpose(np.asarray(W0, np.float32), (1, 0, 2))).astype(bf16)
    w1t = np.ascontiguousarray(
        np.transpose(np.asarray(W1, np.float32), (1, 0, 2))).astype(bf16)
    w2t = np.ascontiguousarray(
        np.transpose(np.asarray(W2, np.float32), (1, 0, 2))).astype(bf16)
    iota = np.broadcast_to(np.arange(P, dtype=np.float32), (P, P))
    ident = np.eye(P, dtype=np.float32)

    in_maps = []
    for c in range(CORES):
        rows = slice(c * SL, (c + 1) * SL)
        in_maps.append(dict(
            xT=np.ascontiguousarray(x_pad[rows].T).astype(bf16),
            xA=xA, xB=xB,
            idxs=pre["idxs"][c],
            mcache=pre["mcache"][c],
            dl=pre["dl"][c],
            dinvd2=pre["dinvd2"][c],
            dinvn=pre["dinvn"][c],
            w0t=w0t, w1t=w1t, w2t=w2t,
            b0=np.asarray(b0, np.float32).reshape(HID, 1),
            b1=np.asarray(b1, np.float32).reshape(HID, 1),
            b2=np.asarray(b2, np.float32).reshape(CLS, 1),
            iota=iota.astype(bf16), identb=ident.astype(bf16), ident=ident,
        ))

    nc = _get_nc(pre)
    kw = {}
    if trace:
        kw = dict(trace=True,
                  trace_cores=trace_cores if trace_cores is not None else [0])
    res = bass_utils.run_bass_kernel_spmd(nc, in_maps,
                                          core_ids=list(range(CORES)), **kw)

    full = np.concatenate([res.results[c]["out"] for c in range(CORES)],
                          axis=0)
    out = full[perm]
    return out.astype(np.float32), res


def kernel(x, edge_src, edge_dst, W0, b0, W1, b1, W2, b2):
    out, _ = _run(x, edge_src, edge_dst, W0, b0, W1, b1, W2, b2)
    return out


# revision 20
# speedup vs baseline: 1.4836x; 1.1138x over previous
"""ChebConv GNN (3 layers, K=4) on 8 Trainium2 NeuronCores.

Node-parallel sharding: an LPT permutation (on in-degree) relabels nodes into
400 windows of 128 dst nodes (50 windows per core). Each SpMM hop gathers
source rows from a replicated node-major HBM table (dma_gather, one 256B
descriptor per edge) and segment-sums them per window with one-hot matmuls on
the TensorEngine.

Key structure vs. a naive implementation:
 - The edge weight -dinv[src]*dinv[dst] is separable: dinv[src] is folded
   into the gather tables (applied when slices are produced), -2*dinv[dst]
   is applied to the segment-sum output per window. The one-hot matrices are
   then static 0/1, identical for all 9 SpMMs: they are host-built in fp8
   and cached in SBUF (matmul takes bf16 lhsT x fp8 rhs), with the overflow
   built on the fly by DVE from compressed dst-slot data.
 - The inter-hop AllGather is split in two (windows 0..29 / 30..49) so the
   first half fires mid-hop; next hop's gathers from the A-half table start
   while the B-half collective is still in flight. A-source gathers lead
   B-source gathers by a few pairs to cover the tail.
 - Everything flows in bf16 (fp32 PSUM accumulation), weights included.
"""

import numpy as np

# ---------------- problem constants (hardcoded per contract) ----------------
N, E = 50000, 800000
F, HID, CLS, K = 128, 128, 40, 4
P = 128
CORES = 8
NW = 50                  # dst windows per core
SL = NW * P              # 6400 nodes per core
NPAD = CORES * SL        # 51200 padded node count
WA = 30                  # windows in the A half (0..29)
WB = NW - WA             # windows in the B half (30..49)
NA = WA * P              # 3840 A-nodes per core
NB = WB * P              # 2560 B-nodes per core
RA = CORES * NA          # 30720 rows in table A (int16-indexable)
RB = CORES * NB          # 20480 rows in table B

LEAD = 3                 # A-gather lead (pairs) over B-gathers
NCHC_MAX = 408           # max one-hot chunks cached in SBUF (fp8)


# ---------------- host preprocessing ----------------
def _lpt_windows(indeg, n_windows, cap):
    """Assign nodes to windows (cap nodes each), balancing in-degree sums.
    Returns perm: old node id -> new node id."""
    import heapq
    order = np.argsort(-indeg, kind="stable")
    heap = [(0, wi) for wi in range(n_windows)]
    heapq.heapify(heap)
    counts = np.zeros(n_windows, np.int64)
    perm = np.empty(len(indeg), np.int64)
    for old in order:
        while True:
            load, wi = heapq.heappop(heap)
            if counts[wi] < cap:
                break
        perm[old] = wi * cap + counts[wi]
        counts[wi] += 1
        if counts[wi] < cap:
            heapq.heappush(heap, (load + int(indeg[old]), wi))
    return perm


def _wrap_idx(flat):
    """Logical idx list [L] -> [128, L//16] wrapped layout for dma_gather."""
    L = flat.shape[0]
    a = flat.reshape(L // 16, 16).T            # [16, L/16]
    return np.tile(a, (8, 1))                  # [128, L/16]


def _preprocess(edge_src, edge_dst, n):
    import ml_dtypes
    bf16 = ml_dtypes.bfloat16
    fp8 = ml_dtypes.float8_e4m3fn

    es = np.asarray(edge_src, np.int64)
    ed = np.asarray(edge_dst, np.int64)
    deg = np.bincount(es, minlength=n).astype(np.float32)
    dinv = np.where(deg > 0, 1.0 / np.sqrt(np.maximum(deg, 1.0)), 0.0).astype(
        np.float32
    )
    indeg = np.bincount(ed, minlength=n)
    perm = _lpt_windows(indeg, CORES * NW, P)      # old -> new
    dinv_new = np.zeros(NPAD, np.float32)
    dinv_new[perm] = dinv

    nsrc = perm[es]
    ndst = perm[ed]
    # source table rows
    score = nsrc // SL
    sloc = nsrc % SL
    half_e = (sloc >= NA).astype(np.int64)         # 0 = A, 1 = B
    srow = np.where(half_e == 0, score * NA + sloc, score * NB + (sloc - NA))
    # dst decomposition
    dcore = ndst // SL
    dl6400 = ndst % SL
    dwin = dl6400 // P
    dloc = dl6400 % P

    # per (core, window, half) counts -> global per-(window, half) chunk caps
    key = (dcore * NW + dwin) * 2 + half_e
    cnt = np.bincount(key, minlength=CORES * NW * 2).reshape(CORES, NW, 2)
    C = np.maximum(1, (cnt.max(axis=0) + P - 1) // P)   # [NW, 2] chunks
    moff = np.zeros((NW, 2), np.int64)                  # chunk-id offsets
    run = 0
    for w in range(NW):
        for h in (0, 1):
            moff[w, h] = run
            run += C[w, h]
    NCH = int(run)

    # slot assignment: rank within (core, window, half)
    order = np.argsort(key, kind="stable")
    key_s = key[order]
    counts_flat = np.bincount(key_s, minlength=CORES * NW * 2)
    starts = np.concatenate([[0], np.cumsum(counts_flat)[:-1]])
    rank = np.arange(len(es)) - starts[key_s]

    ce, we, he = dcore[order], dwin[order], half_e[order]
    de, se, re_ = dloc[order], srow[order], rank

    # build per-core idx arrays, M one-hots, dl (compressed dst-slot)
    idx_cols = int((C[:, 0].sum() + C[:, 1].sum()) * 8)   # NCH * 8
    idxs = np.zeros((CORES, P, idx_cols), np.int16)
    Mfull = np.zeros((CORES, P, NCH, P), np.uint8)
    dl = np.full((CORES, P, NCH), -1.0, np.float32)

    # idx column offsets per (pair, half): pair g covers windows 2g, 2g+1
    nA_pair = (C[0::2, 0] + C[1::2, 0]) * P               # [25]
    nB_pair = (C[0::2, 1] + C[1::2, 1]) * P
    iAoff = np.zeros(NW // 2, np.int64)
    iBoff = np.zeros(NW // 2, np.int64)
    off = 0
    for g in range(NW // 2):
        iAoff[g] = off
        off += nA_pair[g] // 16
        iBoff[g] = off
        off += nB_pair[g] // 16
    assert off == idx_cols

    # flat slot id per edge within its (window, half) block
    chunk_g = moff[we, he] + re_ // P      # global chunk id
    spart = re_ % P                        # slot partition
    Mfull[ce, spart, chunk_g, de] = 1
    dl[ce, spart, chunk_g] = de

    # idx flat arrays: for each (c, g, h): concat windows (2g, 2g+1) blocks,
    # each padded to C[w,h]*128 slots (pad idx 0)
    idx_flat = np.zeros((CORES, NCH * P), np.int64)
    slot_in_blk = moff[we, he] * P + re_
    idx_flat[ce, slot_in_blk] = se
    for c in range(CORES):
        for g in range(NW // 2):
            for h, ioff in ((0, iAoff[g]), (1, iBoff[g])):
                w0, w1 = 2 * g, 2 * g + 1
                blk = np.concatenate([
                    idx_flat[c, moff[w0, h] * P:(moff[w0, h] + C[w0, h]) * P],
                    idx_flat[c, moff[w1, h] * P:(moff[w1, h] + C[w1, h]) * P],
                ])
                wrapped = _wrap_idx(blk.astype(np.int16))
                idxs[c, :, ioff:ioff + blk.shape[0] // 16] = wrapped

    # cache whole windows only, so streamed windows are contiguous runs
    wcut = 0
    while wcut < NW and moff[wcut, 0] + C[wcut, 0] + C[wcut, 1] <= NCHC_MAX:
        wcut += 1
    nchc = int(moff[wcut, 0]) if wcut < NW else NCH
    mcache = np.ascontiguousarray(Mfull).astype(fp8)

    # per-core constants
    dinv_c = dinv_new.reshape(CORES, SL)
    dinvd2 = np.broadcast_to((-2.0 * dinv_c)[:, None, :], (CORES, P, SL))
    dinvn = dinv_c.reshape(CORES, NW, P).transpose(0, 2, 1)  # [c, 128, NW]

    return dict(
        perm=perm, dinv_new=dinv_new, C=C, moff=moff, NCH=NCH, nchc=nchc,
        wcut=wcut,
        nA_pair=nA_pair.astype(np.int64), nB_pair=nB_pair.astype(np.int64),
        iAoff=iAoff, iBoff=iBoff, idxs=idxs, mcache=mcache,
        dinvd2=np.ascontiguousarray(dinvd2.astype(bf16)),
        dinvn=np.ascontiguousarray(dinvn.astype(np.float32)),
        idx_cols=idx_cols,
    )


# ---------------- device kernel ----------------
def _build(sched):
    import concourse.bass as bass
    import concourse.bacc as bacc
    import concourse.tile as tile
    import concourse.mybir as mybir
    import dataclasses

    C = np.asarray(sched["C"], np.int64).reshape(NW, 2)
    moff = np.asarray(sched["moff"], np.int64).reshape(NW, 2)
    NCH = int(sched["NCH"])
    NCHC = int(sched["nchc"])
    nA_pair = np.asarray(sched["nA_pair"], np.int64)
    nB_pair = np.asarray(sched["nB_pair"], np.int64)
    iAoff = np.asarray(sched["iAoff"], np.int64)
    iBoff = np.asarray(sched["iBoff"], np.int64)
    idx_cols = int(sched["idx_cols"])
    WCUT = int(sched["wcut"])
    CAmax = int((C[0::2, 0] + C[1::2, 0]).max())
    CBmax = int((C[0::2, 1] + C[1::2, 1]).max())
    CWmax = int((C[:, 0] + C[:, 1]).max())
    stage = int(sched.get("STAGE", 99))

    fp = mybir.dt.float32
    bf = mybir.dt.bfloat16
    f8 = mybir.dt.float8e4
    Alu = mybir.AluOpType
    Act = mybir.ActivationFunctionType

    nc = bacc.Bacc("TRN2", target_bir_lowering=False, debug=False,
                   num_devices=CORES, num_swdge_queues=4)

    # -------- I/O --------
    xT_d = nc.dram_tensor("xT", [P, SL], bf, kind="ExternalInput")
    xA_d = nc.dram_tensor("xA", [RA, F], bf, kind="ExternalInput")
    xB_d = nc.dram_tensor("xB", [RB, F], bf, kind="ExternalInput")
    idx_d = nc.dram_tensor("idxs", [P, idx_cols], mybir.dt.int16,
                           kind="ExternalInput")
    mc_d = nc.dram_tensor("mcache", [P, NCH, P], f8, kind="ExternalInput")
    dinvd2_d = nc.dram_tensor("dinvd2", [P, SL], bf, kind="ExternalInput")
    dinvn_d = nc.dram_tensor("dinvn", [P, NW], fp, kind="ExternalInput")
    w0_d = nc.dram_tensor("w0t", [P, K, HID], bf, kind="ExternalInput")
    w1_d = nc.dram_tensor("w1t", [P, K, HID], bf, kind="ExternalInput")
    w2_d = nc.dram_tensor("w2t", [P, K, CLS], bf, kind="ExternalInput")
    b0_d = nc.dram_tensor("b0", [HID, 1], fp, kind="ExternalInput")
    b1_d = nc.dram_tensor("b1", [HID, 1], fp, kind="ExternalInput")
    b2_d = nc.dram_tensor("b2", [CLS, 1], fp, kind="ExternalInput")
    identb_d = nc.dram_tensor("identb", [P, P], bf, kind="ExternalInput")
    ident_d = nc.dram_tensor("ident", [P, P], fp, kind="ExternalInput")
    out_d = nc.dram_tensor("out", [SL, CLS], fp, kind="ExternalOutput")

    def bmid(ap, n):  # [128, X] -> [128, n, X], middle stride 0
        return dataclasses.replace(ap, ap=[ap.ap[0], [0, n], ap.ap[1]])

    def blast(ap, n):  # [128, X] -> [128, X, n], last stride 0
        return dataclasses.replace(ap, ap=[ap.ap[0], ap.ap[1], [0, n]])

    qctr = [0]

    def nxtq():
        qctr[0] = (qctr[0] + 1) % 4
        return qctr[0]

    with tile.TileContext(nc) as tc:
        with (
            tc.tile_pool(name="const", bufs=1) as constp,
            tc.tile_pool(name="tx", bufs=4) as txp,
            tc.tile_pool(name="acc", bufs=1) as accp,
            tc.tile_pool(name="gA", bufs=LEAD + 2) as gAp,
            tc.tile_pool(name="gB", bufs=2) as gBp,
            tc.tile_pool(name="tmp", bufs=2) as tmpp,
            tc.tile_pool(name="ms", bufs=4) as msp,
            tc.tile_pool(name="st", bufs=4) as stp,
            tc.tile_pool(name="psA", bufs=2, space="PSUM") as psA,
            tc.tile_pool(name="psT", bufs=2, space="PSUM") as psT,
            tc.tile_pool(name="psW", bufs=2, space="PSUM") as psW,
            tc.tile_pool(name="slA", bufs=2, space="DRAM") as slAp,
            tc.tile_pool(name="slB", bufs=2, space="DRAM") as slBp,
            tc.tile_pool(name="tabA", bufs=2, space="DRAM") as tabAp,
            tc.tile_pool(name="tabB", bufs=2, space="DRAM") as tabBp,
        ):
            # -------- constants --------
            mc_t = constp.tile([P, NCHC, P], f8)
            idx_t = constp.tile([P, idx_cols], mybir.dt.int16)
            dinvd2_t = constp.tile([P, SL], bf)
            dinvn_t = constp.tile([P, NW], fp)
            identb_t = constp.tile([P, P], bf)
            ident_t = constp.tile([P, P], fp)
            w0_t = constp.tile([P, K, HID], bf)
            w1_t = constp.tile([P, K, HID], bf)
            w2_t = constp.tile([P, K, CLS], bf)
            b0_t = constp.tile([HID, 1], fp)
            b1_t = constp.tile([HID, 1], fp)
            b2_t = constp.tile([CLS, 1], fp)
            for t, d in ((idx_t, idx_d),
                         (dinvd2_t, dinvd2_d), (dinvn_t, dinvn_d),
                         (identb_t, identb_d),
                         (ident_t, ident_d),
                         (w0_t, w0_d), (w1_t, w1_d), (w2_t, w2_d),
                         (b0_t, b0_d), (b1_t, b1_d), (b2_t, b2_d)):
                nc.sync.dma_start(out=t[:], in_=d[:])

            if NCHC > 0:
                nc.sync.dma_start(out=mc_t[:], in_=mc_d[:, :NCHC, :])
            tx0 = txp.tile([P, SL], bf, tag="tx")
            nc.sync.dma_start(out=tx0[:], in_=xT_d[:, :])

            def spmm(tabA_ap, tabB_ap, tx_prev2, Wt, fo, acc, k, hctx):
                """One lhat hop. hctx = (last_layer, hT, sliceA, sliceB,
                dinvn) context for the k==3 fused epilogue."""
                last, hT, slA_t, slB_t = hctx
                tx_new = txp.tile([P, SL], bf, tag="tx")
                mk_slice = k < 3
                ga = {}

                def issue_A(g):
                    t = gAp.tile([P, CAmax, P], bf, tag="GA")
                    ca = int((nA_pair[g]) // P)
                    nc.gpsimd.dma_gather(
                        out_ap=t[:, :ca, :], in_ap=tabA_ap,
                        idxs_ap=idx_t[:, iAoff[g]:iAoff[g] + nA_pair[g] // 16],
                        num_idxs=int(nA_pair[g]), num_idxs_reg=int(nA_pair[g]),
                        elem_size=P, single_packet=False, queue_num=nxtq())
                    ga[g] = t

                for g in range(min(LEAD, NW // 2)):
                    issue_A(g)

                for g in range(NW // 2):
                    if g + LEAD < NW // 2:
                        issue_A(g + LEAD)
                    mst = {}
                    for wloc in (0, 1):
                        w = 2 * g + wloc
                        if w >= WCUT:
                            cw = int(C[w, 0] + C[w, 1])
                            mo = int(moff[w, 0])
                            mt = msp.tile([P, CWmax, P], f8, tag="ms",
                                          name="mst_w")
                            nc.sync.dma_start(out=mt[:, :cw, :],
                                              in_=mc_d[:, mo:mo + cw, :])
                            mst[w] = (mt, mo)
                    gb = gBp.tile([P, CBmax, P], bf, tag="GB")
                    cb = int(nB_pair[g] // P)
                    nc.gpsimd.dma_gather(
                        out_ap=gb[:, :cb, :], in_ap=tabB_ap,
                        idxs_ap=idx_t[:, iBoff[g]:iBoff[g] + nB_pair[g] // 16],
                        num_idxs=int(nB_pair[g]), num_idxs_reg=int(nB_pair[g]),
                        elem_size=P, single_packet=False, queue_num=nxtq())
                    gat = ga.pop(g)
                    aoff = 0 if True else 0
                    for wloc in (0, 1):
                        w = 2 * g + wloc
                        wb = slice(w * P, (w + 1) * P)
                        ps = psA.tile([P, P], fp, tag="ps")
                        na0 = int(C[2 * g, 0])
                        nb0 = int(C[2 * g, 1])
                        chunks = []
                        if wloc == 0:
                            chunks += [(gat, i, int(moff[w, 0]) + i)
                                       for i in range(int(C[w, 0]))]
                            chunks += [(gb, i, int(moff[w, 1]) + i)
                                       for i in range(int(C[w, 1]))]
                        else:
                            chunks += [(gat, na0 + i, int(moff[w, 0]) + i)
                                       for i in range(int(C[w, 0]))]
                            chunks += [(gb, nb0 + i, int(moff[w, 1]) + i)
                                       for i in range(int(C[w, 1]))]
                        nchk = len(chunks)
                        for j, (buf, lc, ms) in enumerate(chunks):
                            if ms < NCHC:
                                rhs = mc_t[:, ms, :]
                            else:
                                mt, mo = mst[w]
                                rhs = mt[:, ms - mo, :]
                            nc.tensor.matmul(
                                out=ps[:], lhsT=buf[:, lc, :], rhs=rhs,
                                start=(j == 0), stop=(j == nchk - 1))
                        tmp = tmpp.tile([P, P], fp, tag="tmp")
                        nc.vector.tensor_tensor(
                            out=tmp[:], in0=ps[:], in1=dinvd2_t[:, wb],
                            op=Alu.mult)
                        if k == 1:
                            nc.vector.tensor_scalar(
                                out=tx_new[:, wb], in0=tmp[:], scalar1=0.5,
                                scalar2=None, op0=Alu.mult)
                        else:
                            nc.vector.tensor_tensor(
                                out=tx_new[:, wb], in0=tmp[:],
                                in1=tx_prev2[:, wb], op=Alu.subtract)
                        psw = psW.tile([P, P], fp, tag="psw")
                        nc.tensor.matmul(out=psw[:fo, :], lhsT=Wt[:, k, :fo],
                                         rhs=tx_new[:, wb], start=True,
                                         stop=True)
                        nc.vector.tensor_tensor(out=acc[:fo, wb],
                                                in0=acc[:fo, wb],
                                                in1=psw[:fo, :], op=Alu.add)
                        if mk_slice:
                            pst = psT.tile([P, P], bf, tag="pst")
                            nc.tensor.transpose(out=pst[:], in_=tx_new[:, wb],
                                                identity=identb_t[:])
                            st = stp.tile([P, P], bf, tag="st")
                            nc.scalar.activation(out=st[:], in_=pst[:],
                                                 func=Act.Copy,
                                                 scale=dinvn_t[:, w:w + 1])
                            if w < WA:
                                nc.scalar.dma_start(
                                    out=slA_t[w * P:(w + 1) * P, :], in_=st[:])
                            else:
                                nc.scalar.dma_start(
                                    out=slB_t[(w - WA) * P:(w - WA + 1) * P, :],
                                    in_=st[:])
                        elif not last:
                            # k == 3: finish acc, produce h slice + hT
                            nc.scalar.activation(out=hT[:, wb],
                                                 in_=acc[:, wb], func=Act.Relu)
                            pst = psT.tile([P, P], bf, tag="pst")
                            nc.tensor.transpose(out=pst[:], in_=hT[:, wb],
                                                identity=identb_t[:])
                            st = stp.tile([P, P], bf, tag="st")
                            nc.scalar.activation(out=st[:], in_=pst[:],
                                                 func=Act.Copy,
                                                 scale=dinvn_t[:, w:w + 1])
                            if w < WA:
                                nc.scalar.dma_start(
                                    out=slA_t[w * P:(w + 1) * P, :], in_=st[:])
                            else:
                                nc.scalar.dma_start(
                                    out=slB_t[(w - WA) * P:(w - WA + 1) * P, :],
                                    in_=st[:])
                        else:
                            # k == 3, last layer: log_softmax epilogue
                            pst = psT.tile([P, P], fp, tag="pst32")
                            nc.tensor.transpose(out=pst[:, :CLS],
                                                in_=acc[:CLS, wb],
                                                identity=ident_t[:CLS, :CLS])
                            nm = stp.tile([P, 1], fp, tag="nm")
                            nc.vector.tensor_reduce(
                                out=nm[:], in_=pst[:, :CLS], op=Alu.max,
                                axis=mybir.AxisListType.X, negate=True)
                            ex = stp.tile([P, CLS], fp, tag="ex")
                            ssum = stp.tile([P, 1], fp, tag="ssum")
                            nc.scalar.activation(out=ex[:], in_=pst[:, :CLS],
                                                 func=Act.Exp, bias=nm[:, 0:1],
                                                 accum_out=ssum[:, 0:1])
                            lse = stp.tile([P, 1], fp, tag="lse")
                            nc.scalar.activation(out=lse[:], in_=ssum[:],
                                                 func=Act.Ln)
                            res = stp.tile([P, CLS], fp, tag="res")
                            nc.vector.tensor_scalar(
                                out=res[:], in0=pst[:, :CLS],
                                scalar1=nm[:, 0:1], scalar2=lse[:, 0:1],
                                op0=Alu.add, op1=Alu.subtract)
                            nc.scalar.dma_start(out=out_d[w * P:(w + 1) * P, :],
                                                in_=res[:])
                    # fire the A-half collective once windows 0..WA-1 done
                    if g == WA // 2 - 1 and (mk_slice or not last):
                        tabA_new = tabAp.tile([RA, F], bf, tag="tabA",
                                              addr_space="Shared")
                        nc.gpsimd.collective_compute(
                            "AllGather", Alu.bypass,
                            replica_groups=[list(range(CORES))],
                            ins=[slA_t[:, :].opt()],
                            outs=[tabA_new[:, :].opt()])
                        hctx2 = tabA_new
                    elif g == WA // 2 - 1:
                        hctx2 = None
                if mk_slice or not last:
                    tabB_new = tabBp.tile([RB, F], bf, tag="tabB",
                                          addr_space="Shared")
                    nc.gpsimd.collective_compute(
                        "AllGather", Alu.bypass,
                        replica_groups=[list(range(CORES))],
                        ins=[slB_t[:, :].opt()],
                        outs=[tabB_new[:, :].opt()])
                    return tx_new, hctx2, tabB_new
                return tx_new, None, None

            tabA_cur = xA_d[0:RA, :]
            tabB_cur = xB_d[0:RB, :]
            for l, (Wt, b_t, fo) in enumerate(
                    ((w0_t, b0_t, HID), (w1_t, b1_t, HID), (w2_t, b2_t, CLS))):
                if l * 10 >= stage:
                    break
                last = l == 2
                acc = accp.tile([P, SL], fp, tag="acc")
                # ---- k=0 term ----
                for w in range(NW):
                    wb = slice(w * P, (w + 1) * P)
                    psw = psW.tile([P, P], fp, tag="psw")
                    nc.tensor.matmul(out=psw[:fo, :], lhsT=Wt[:, 0, :fo],
                                     rhs=tx0[:, wb], start=True, stop=True)
                    nc.vector.tensor_scalar(
                        out=acc[:fo, wb], in0=psw[:fo, :],
                        scalar1=b_t[:fo, 0:1], scalar2=None, op0=Alu.add)
                # ---- hops ----
                hT = (None if last
                      else txp.tile([P, SL], bf, tag="tx", name="hT"))
                tx1 = tx2 = tx3 = None
                for k in (1, 2, 3):
                    if stage < l * 10 + k + 1:
                        break
                    mk_slice = k < 3
                    slA_t = (slAp.tile([NA, F], bf, tag="slA", name="slA_t")
                             if (mk_slice or not last) else None)
                    slB_t = (slBp.tile([NB, F], bf, tag="slB", name="slB_t")
                             if (mk_slice or not last) else None)
                    prev2 = None if k == 1 else (tx0 if k == 2 else tx1)
                    txn, tA, tB = spmm(tabA_cur, tabB_cur, prev2, Wt, fo, acc,
                                       k, (last, hT, slA_t, slB_t))
                    if k == 1:
                        tx1 = txn
                    elif k == 2:
                        tx2 = txn
                    else:
                        tx3 = txn
                    if tA is not None:
                        tabA_cur = tA[0:RA, :]
                        tabB_cur = tB[0:RB, :]
                if not last:
                    tx0 = hT

    nc.compile()
    return nc


_CACHE = {}


def _get_nc(sched):
    key = (tuple(np.asarray(sched["C"]).flatten().tolist()),
           sched["NCH"], sched["nchc"], sched["wcut"],
           sched.get("STAGE", 99))
    if key not in _CACHE:
        _CACHE[key] = _build(sched)
    return _CACHE[key]


def _run(x, edge_src, edge_dst, W0, b0, W1, b1, W2, b2, cfg=None,
         trace=False, trace_cores=None):
    from concourse import bass_utils
    import ml_dtypes
    bf16 = ml_dtypes.bfloat16

    n = x.shape[0]
    pre = _preprocess(edge_src, edge_dst, n)
    if cfg and "STAGE" in cfg:
        pre["STAGE"] = cfg["STAGE"]
    perm = pre["perm"]

    x = np.asarray(x, np.float32)
    x_pad = np.zeros((NPAD, F), np.float32)
    x_pad[perm] = x
    xs = x_pad * pre["dinv_new"][:, None]          # dinv-scaled rows
    xs_c = xs.reshape(CORES, NW, P, F)
    xA = np.ascontiguousarray(
        xs_c[:, :WA].reshape(CORES * NA, F)).astype(bf16)
    xB = np.ascontiguousarray(
        xs_c[:, WA:].reshape(CORES * NB, F)).astype(bf16)

    w0t = np.ascontiguousarray(
        np.transpose(np.asarray(W0, np.float32), (1, 0, 2))).astype(bf16)
    w1t = np.ascontiguousarray(
        np.transpose(np.asarray(W1, np.float32), (1, 0, 2))).astype(bf16)
    w2t = np.ascontiguousarray(
        np.transpose(np.asarray(W2, np.float32), (1, 0, 2))).astype(bf16)
    iota = np.broadcast_to(np.arange(P, dtype=np.float32), (P, P))
    ident = np.eye(P, dtype=np.float32)

    in_maps = []
    for c in range(CORES):
        rows = slice(c * SL, (c + 1) * SL)
        in_maps.append(dict(
            xT=np.ascontiguousarray(x_pad[rows].T).astype(bf16),
            xA=xA, xB=xB,
            idxs=pre["idxs"][c],
            mcache=pre["mcache"][c],
            dinvd2=pre["dinvd2"][c],
            dinvn=pre["dinvn"][c],
            w0t=w0t, w1t=w1t, w2t=w2t,
            b0=np.asarray(b0, np.float32).reshape(HID, 1),
            b1=np.asarray(b1, np.float32).reshape(HID, 1),
            b2=np.asarray(b2, np.float32).reshape(CLS, 1),
            identb=ident.astype(bf16), ident=ident,
        ))

    nc = _get_nc(pre)
    kw = {}
    if trace:
        kw = dict(trace=True,
                  trace_cores=trace_cores if trace_cores is not None else [0])
    res = bass_utils.run_bass_kernel_spmd(nc, in_maps,
                                          core_ids=list(range(CORES)), **kw)

    full = np.concatenate([res.results[c]["out"] for c in range(CORES)],
                          axis=0)
    out = full[perm]
    return out.astype(np.float32), res


def kernel(x, edge_src, edge_dst, W0, b0, W1, b1, W2, b2):
    out, _ = _run(x, edge_src, edge_dst, W0, b0, W1, b1, W2, b2)
    return out


# revision 32
# speedup vs baseline: 1.5332x; 1.0334x over previous
"""ChebConv GNN (3 layers, K=4) on 8 Trainium2 NeuronCores.

Node-parallel sharding: an LPT permutation (on in-degree) relabels nodes into
400 windows of 128 dst nodes (50 windows per core). Each SpMM hop gathers
source rows from a replicated node-major HBM table (dma_gather, one 256B
descriptor per edge) and segment-sums them per window with one-hot matmuls on
the TensorEngine.

Key structure vs. a naive implementation:
 - The edge weight -dinv[src]*dinv[dst] is separable: dinv[src] is folded
   into the gather tables (applied when slices are produced), -2*dinv[dst]
   is applied to the segment-sum output per window. The one-hot matrices are
   then static 0/1, identical for all 9 SpMMs: they are host-built in fp8
   and cached in SBUF (matmul takes bf16 lhsT x fp8 rhs), with the overflow
   built on the fly by DVE from compressed dst-slot data.
 - The inter-hop AllGather is split in two (windows 0..29 / 30..49) so the
   first half fires mid-hop; next hop's gathers from the A-half table start
   while the B-half collective is still in flight. A-source gathers lead
   B-source gathers by a few pairs to cover the tail.
 - Everything flows in bf16 (fp32 PSUM accumulation), weights included.
"""

import numpy as np

# ---------------- problem constants (hardcoded per contract) ----------------
N, E = 50000, 800000
F, HID, CLS, K = 128, 128, 40, 4
P = 128
CORES = 8
NW = 50                  # dst windows per core
SL = NW * P              # 6400 nodes per core
NPAD = CORES * SL        # 51200 padded node count
WA = 28                  # windows in the A half (0..27)
WB = NW - WA             # windows in the B half (28..49)
NA = WA * P              # A-nodes per core
NB = WB * P              # B-nodes per core
RA = CORES * NA          # 28672 rows in table A (int16-indexable)
RB = CORES * NB          # 22528 rows in table B

GQ = 2                   # dst windows per gather group
GROUPS = [(s, min(s + GQ, NW)) for s in range(0, NW, GQ)]  # 25 groups
LEADG = 3                # A-gather lead (groups) over B-gathers
NCHC_MAX = 400           # max one-hot chunks cached in SBUF (fp8)


# ---------------- host preprocessing ----------------
def _lpt_windows(indeg, n_windows, cap):
    """Assign nodes to windows (cap nodes each), balancing in-degree sums.
    Returns perm: old node id -> new node id."""
    import heapq
    order = np.argsort(-indeg, kind="stable")
    heap = [(0, wi) for wi in range(n_windows)]
    heapq.heapify(heap)
    counts = np.zeros(n_windows, np.int64)
    perm = np.empty(len(indeg), np.int64)
    for old in order:
        while True:
            load, wi = heapq.heappop(heap)
            if counts[wi] < cap:
                break
        perm[old] = wi * cap + counts[wi]
        counts[wi] += 1
        if counts[wi] < cap:
            heapq.heappush(heap, (load + int(indeg[old]), wi))
    return perm


def _wrap_idx(flat):
    """Logical idx list [L] -> [128, L//16] wrapped layout for dma_gather."""
    L = flat.shape[0]
    a = flat.reshape(L // 16, 16).T            # [16, L/16]
    return np.tile(a, (8, 1))                  # [128, L/16]


def _preprocess(edge_src, edge_dst, n):
    import ml_dtypes
    bf16 = ml_dtypes.bfloat16
    fp8 = ml_dtypes.float8_e4m3fn

    es = np.asarray(edge_src, np.int64)
    ed = np.asarray(edge_dst, np.int64)
    deg = np.bincount(es, minlength=n).astype(np.float32)
    dinv = np.where(deg > 0, 1.0 / np.sqrt(np.maximum(deg, 1.0)), 0.0).astype(
        np.float32
    )
    indeg = np.bincount(ed, minlength=n)
    perm = _lpt_windows(indeg, CORES * NW, P)      # old -> new
    dinv_new = np.zeros(NPAD, np.float32)
    dinv_new[perm] = dinv

    nsrc = perm[es]
    ndst = perm[ed]
    # source table rows
    score = nsrc // SL
    sloc = nsrc % SL
    half_e = (sloc >= NA).astype(np.int64)         # 0 = A, 1 = B
    srow = np.where(half_e == 0, score * NA + sloc, score * NB + (sloc - NA))
    # dst decomposition
    dcore = ndst // SL
    dl6400 = ndst % SL
    dwin = dl6400 // P
    dloc = dl6400 % P

    # dedup edges sharing (core, window, half, src): one gathered slot can
    # feed multiple dst columns of its window's one-hot chunk
    key = (dcore * NW + dwin) * 2 + half_e
    uord = np.lexsort((srow, key))
    ku, su, du = key[uord], srow[uord], dloc[uord]
    new_u = np.ones(len(es), bool)
    new_u[1:] = (ku[1:] != ku[:-1]) | (su[1:] != su[:-1])
    gs_idx = np.flatnonzero(np.concatenate([[True], ku[1:] != ku[:-1]]))
    grp_of = np.cumsum(np.concatenate([[True], ku[1:] != ku[:-1]])) - 1
    uid = np.cumsum(new_u) - 1                      # global unique-slot id
    uid_base_of_grp = uid[gs_idx]
    rank = uid - uid_base_of_grp[grp_of]            # unique rank in group

    # per (core, window, half) unique counts
    ukey = ku[new_u]
    cnt = np.bincount(ukey, minlength=CORES * NW * 2).reshape(CORES, NW, 2)
    C = np.maximum(1, (cnt.max(axis=0) + P - 1) // P)   # [NW, 2] chunks
    moff = np.zeros((NW, 2), np.int64)                  # chunk-id offsets
    run = 0
    for w in range(NW):
        for h in (0, 1):
            moff[w, h] = run
            run += C[w, h]
    NCH = int(run)

    ce = ku // (NW * 2)
    we = (ku // 2) % NW
    he = ku % 2

    # build per-core idx arrays and M one-hots
    Mfull = np.zeros((CORES, P, NCH, P), np.uint8)
    chunk_g = moff[we, he] + rank // P      # global chunk id (per edge)
    spart = rank % P                        # slot partition
    np.add.at(Mfull, (ce, spart, chunk_g, du), 1)

    # idx flat arrays (per unique slot)
    idx_flat = np.zeros((CORES, NCH * P), np.int64)
    slot_in_blk = moff[we, he] * P + rank
    idx_flat[ce[new_u], slot_in_blk[new_u]] = su[new_u]

    # idx column layout per (group, half)
    nA_grp = np.array([C[s:e, 0].sum() * P for s, e in GROUPS], np.int64)
    nB_grp = np.array([C[s:e, 1].sum() * P for s, e in GROUPS], np.int64)
    iAoff = np.zeros(len(GROUPS), np.int64)
    iBoff = np.zeros(len(GROUPS), np.int64)
    off = 0
    for g in range(len(GROUPS)):
        iAoff[g] = off
        off += nA_grp[g] // 16
        iBoff[g] = off
        off += nB_grp[g] // 16
    idx_cols = int(off)
    idxs = np.zeros((CORES, P, idx_cols), np.int16)
    for c in range(CORES):
        for g, (ws, wend) in enumerate(GROUPS):
            for h, ioff in ((0, iAoff[g]), (1, iBoff[g])):
                blk = np.concatenate([
                    idx_flat[c, moff[w, h] * P:(moff[w, h] + C[w, h]) * P]
                    for w in range(ws, wend)])
                wrapped = _wrap_idx(blk.astype(np.int16))
                idxs[c, :, ioff:ioff + blk.shape[0] // 16] = wrapped

    # cache whole windows only, so streamed windows are contiguous runs
    wcut = 0
    while wcut < NW and moff[wcut, 0] + C[wcut, 0] + C[wcut, 1] <= NCHC_MAX:
        wcut += 1
    nchc = int(moff[wcut, 0]) if wcut < NW else NCH
    mcache = np.ascontiguousarray(Mfull).astype(fp8)

    # per-core constants
    dinv_c = dinv_new.reshape(CORES, SL)
    dinvd2 = np.broadcast_to((-2.0 * dinv_c)[:, None, :], (CORES, P, SL))
    dinvn = dinv_c.reshape(CORES, NW, P).transpose(0, 2, 1)  # [c, 128, NW]

    return dict(
        perm=perm, dinv_new=dinv_new, C=C, moff=moff, NCH=NCH, nchc=nchc,
        wcut=wcut,
        nA_grp=nA_grp, nB_grp=nB_grp,
        iAoff=iAoff, iBoff=iBoff, idxs=idxs, mcache=mcache,
        dinvd2=np.ascontiguousarray(dinvd2.astype(bf16)),
        dinvn=np.ascontiguousarray(dinvn.astype(np.float32)),
        idx_cols=idx_cols,
    )


# ---------------- device kernel ----------------
def _build(sched):
    import concourse.bass as bass
    import concourse.bacc as bacc
    import concourse.tile as tile
    import concourse.mybir as mybir
    import dataclasses

    C = np.asarray(sched["C"], np.int64).reshape(NW, 2)
    moff = np.asarray(sched["moff"], np.int64).reshape(NW, 2)
    NCH = int(sched["NCH"])
    NCHC = int(sched["nchc"])
    idx_cols = int(sched["idx_cols"])
    WCUT = int(sched["wcut"])
    nA_grp = np.asarray(sched["nA_grp"], np.int64)
    nB_grp = np.asarray(sched["nB_grp"], np.int64)
    iAoff = np.asarray(sched["iAoff"], np.int64)
    iBoff = np.asarray(sched["iBoff"], np.int64)
    CAmax = int(nA_grp.max()) // P
    CBmax = int(nB_grp.max()) // P
    CWmax = int((C[:, 0] + C[:, 1]).max())
    NG = len(GROUPS)
    agA_g = next(g for g, (s, e) in enumerate(GROUPS) if e == WA)
    stage = int(sched.get("STAGE", 99))

    fp = mybir.dt.float32
    bf = mybir.dt.bfloat16
    f8 = mybir.dt.float8e4
    Alu = mybir.AluOpType
    Act = mybir.ActivationFunctionType

    nc = bacc.Bacc("TRN2", target_bir_lowering=False, debug=False,
                   num_devices=CORES, num_swdge_queues=4)

    # -------- I/O --------
    xT_d = nc.dram_tensor("xT", [P, SL], bf, kind="ExternalInput")
    xA_d = nc.dram_tensor("xA", [RA, F], bf, kind="ExternalInput")
    xB_d = nc.dram_tensor("xB", [RB, F], bf, kind="ExternalInput")
    idx_d = nc.dram_tensor("idxs", [P, idx_cols], mybir.dt.int16,
                           kind="ExternalInput")
    mc_d = nc.dram_tensor("mcache", [P, NCH, P], f8, kind="ExternalInput")
    dinvd2_d = nc.dram_tensor("dinvd2", [P, SL], bf, kind="ExternalInput")
    dinvn_d = nc.dram_tensor("dinvn", [P, NW], fp, kind="ExternalInput")
    w0_d = nc.dram_tensor("w0t", [P, K, HID], bf, kind="ExternalInput")
    w1_d = nc.dram_tensor("w1t", [P, K, HID], bf, kind="ExternalInput")
    w2_d = nc.dram_tensor("w2t", [P, K, CLS], bf, kind="ExternalInput")
    b0_d = nc.dram_tensor("b0", [HID, 1], fp, kind="ExternalInput")
    b1_d = nc.dram_tensor("b1", [HID, 1], fp, kind="ExternalInput")
    b2_d = nc.dram_tensor("b2", [CLS, 1], fp, kind="ExternalInput")
    identb_d = nc.dram_tensor("identb", [P, P], bf, kind="ExternalInput")
    ident_d = nc.dram_tensor("ident", [P, P], fp, kind="ExternalInput")
    out_d = nc.dram_tensor("out", [SL, CLS], fp, kind="ExternalOutput")

    def bmid(ap, n):  # [128, X] -> [128, n, X], middle stride 0
        return dataclasses.replace(ap, ap=[ap.ap[0], [0, n], ap.ap[1]])

    def blast(ap, n):  # [128, X] -> [128, X, n], last stride 0
        return dataclasses.replace(ap, ap=[ap.ap[0], ap.ap[1], [0, n]])

    qctr = [0]

    def nxtq():
        qctr[0] = (qctr[0] + 1) % 4
        return qctr[0]

    with tile.TileContext(nc) as tc:
        with (
            tc.tile_pool(name="const", bufs=1) as constp,
            tc.tile_pool(name="tx", bufs=4) as txp,
            tc.tile_pool(name="acc", bufs=1) as accp,
            tc.tile_pool(name="gA", bufs=LEADG + 2) as gAp,
            tc.tile_pool(name="gB", bufs=2) as gBp,
            tc.tile_pool(name="tmp", bufs=2) as tmpp,
            tc.tile_pool(name="ms", bufs=4) as msp,
            tc.tile_pool(name="st", bufs=4) as stp,
            tc.tile_pool(name="psA", bufs=2, space="PSUM") as psA,
            tc.tile_pool(name="psT", bufs=2, space="PSUM") as psT,
            tc.tile_pool(name="psW", bufs=2, space="PSUM") as psW,
            tc.tile_pool(name="slA", bufs=2, space="DRAM") as slAp,
            tc.tile_pool(name="slB", bufs=2, space="DRAM") as slBp,
            tc.tile_pool(name="tabA", bufs=2, space="DRAM") as tabAp,
            tc.tile_pool(name="tabB", bufs=2, space="DRAM") as tabBp,
        ):
            # -------- constants --------
            mc_t = constp.tile([P, NCHC, P], f8)
            idx_t = constp.tile([P, idx_cols], mybir.dt.int16)
            dinvd2_t = constp.tile([P, SL], bf)
            dinvn_t = constp.tile([P, NW], fp)
            identb_t = constp.tile([P, P], bf)
            ident_t = constp.tile([P, P], fp)
            w0_t = constp.tile([P, K, HID], bf)
            w1_t = constp.tile([P, K, HID], bf)
            w2_t = constp.tile([P, K, CLS], bf)
            b0_t = constp.tile([HID, 1], fp)
            b1_t = constp.tile([HID, 1], fp)
            b2_t = constp.tile([CLS, 1], fp)
            for t, d in ((idx_t, idx_d),
                         (dinvd2_t, dinvd2_d), (dinvn_t, dinvn_d),
                         (identb_t, identb_d),
                         (ident_t, ident_d),
                         (w0_t, w0_d), (w1_t, w1_d), (w2_t, w2_d),
                         (b0_t, b0_d), (b1_t, b1_d), (b2_t, b2_d)):
                nc.sync.dma_start(out=t[:], in_=d[:])

            if NCHC > 0:
                nc.sync.dma_start(out=mc_t[:], in_=mc_d[:, :NCHC, :])
            tx0 = txp.tile([P, SL], bf, tag="tx")
            nc.sync.dma_start(out=tx0[:], in_=xT_d[:, :])

            def spmm(tabA_ap, tabB_ap, tx_prev2, Wt, fo, acc, k, hctx):
                """One lhat hop. hctx = (last_layer, hT, sliceA, sliceB,
                dinvn) context for the k==3 fused epilogue."""
                last, hT, slA_t, slB_t = hctx
                tx_new = txp.tile([P, SL], bf, tag="tx")
                mk_slice = k < 3
                ga = {}

                def issue_A(g):
                    t = gAp.tile([P, CAmax, P], bf, tag="GA")
                    ca = int(nA_grp[g]) // P
                    nc.gpsimd.dma_gather(
                        out_ap=t[:, :ca, :], in_ap=tabA_ap,
                        idxs_ap=idx_t[:, iAoff[g]:iAoff[g] + nA_grp[g] // 16],
                        num_idxs=int(nA_grp[g]), num_idxs_reg=int(nA_grp[g]),
                        elem_size=P, single_packet=False, queue_num=nxtq())
                    ga[g] = t

                for g in range(min(LEADG, NG)):
                    issue_A(g)

                for g, (ws, wend) in enumerate(GROUPS):
                    if g + LEADG < NG:
                        issue_A(g + LEADG)
                    mst = {}
                    for w in range(ws, wend):
                        if w >= WCUT:
                            cw = int(C[w, 0] + C[w, 1])
                            mo = int(moff[w, 0])
                            mt = msp.tile([P, CWmax, P], f8, tag="ms",
                                          name="mst_w")
                            nc.sync.dma_start(out=mt[:, :cw, :],
                                              in_=mc_d[:, mo:mo + cw, :])
                            mst[w] = (mt, mo)
                    gb = gBp.tile([P, CBmax, P], bf, tag="GB")
                    cb = int(nB_grp[g]) // P
                    nc.gpsimd.dma_gather(
                        out_ap=gb[:, :cb, :], in_ap=tabB_ap,
                        idxs_ap=idx_t[:, iBoff[g]:iBoff[g] + nB_grp[g] // 16],
                        num_idxs=int(nB_grp[g]), num_idxs_reg=int(nB_grp[g]),
                        elem_size=P, single_packet=False, queue_num=nxtq())
                    gat = ga.pop(g)
                    aoff = 0
                    boff = 0
                    for w in range(ws, wend):
                        wb = slice(w * P, (w + 1) * P)
                        ps = psA.tile([P, P], fp, tag="ps")
                        chunks = (
                            [(gat, aoff + i, int(moff[w, 0]) + i)
                             for i in range(int(C[w, 0]))]
                            + [(gb, boff + i, int(moff[w, 1]) + i)
                               for i in range(int(C[w, 1]))])
                        aoff += int(C[w, 0])
                        boff += int(C[w, 1])
                        nchk = len(chunks)
                        for j, (buf, lc, ms) in enumerate(chunks):
                            if ms < NCHC:
                                rhs = mc_t[:, ms, :]
                            else:
                                mt, mo = mst[w]
                                rhs = mt[:, ms - mo, :]
                            nc.tensor.matmul(
                                out=ps[:], lhsT=buf[:, lc, :], rhs=rhs,
                                start=(j == 0), stop=(j == nchk - 1))
                        tmp = tmpp.tile([P, P], fp, tag="tmp")
                        nc.vector.tensor_tensor(
                            out=tmp[:], in0=ps[:], in1=dinvd2_t[:, wb],
                            op=Alu.mult)
                        if k == 1:
                            nc.vector.tensor_scalar(
                                out=tx_new[:, wb], in0=tmp[:], scalar1=0.5,
                                scalar2=None, op0=Alu.mult)
                        else:
                            nc.vector.tensor_tensor(
                                out=tx_new[:, wb], in0=tmp[:],
                                in1=tx_prev2[:, wb], op=Alu.subtract)
                        psw = psW.tile([P, P], fp, tag="psw")
                        nc.tensor.matmul(out=psw[:fo, :], lhsT=Wt[:, k, :fo],
                                         rhs=tx_new[:, wb], start=True,
                                         stop=True)
                        nc.vector.tensor_tensor(out=acc[:fo, wb],
                                                in0=acc[:fo, wb],
                                                in1=psw[:fo, :], op=Alu.add)
                        if mk_slice:
                            pst = psT.tile([P, P], bf, tag="pst")
                            nc.tensor.transpose(out=pst[:], in_=tx_new[:, wb],
                                                identity=identb_t[:])
                            st = stp.tile([P, P], bf, tag="st")
                            nc.scalar.activation(out=st[:], in_=pst[:],
                                                 func=Act.Copy,
                                                 scale=dinvn_t[:, w:w + 1])
                            if w < WA:
                                nc.scalar.dma_start(
                                    out=slA_t[w * P:(w + 1) * P, :], in_=st[:])
                            else:
                                nc.scalar.dma_start(
                                    out=slB_t[(w - WA) * P:(w - WA + 1) * P, :],
                                    in_=st[:])
                        elif not last:
                            # k == 3: finish acc, produce h slice + hT
                            nc.scalar.activation(out=hT[:, wb],
                                                 in_=acc[:, wb], func=Act.Relu)
                            pst = psT.tile([P, P], bf, tag="pst")
                            nc.tensor.transpose(out=pst[:], in_=hT[:, wb],
                                                identity=identb_t[:])
                            st = stp.tile([P, P], bf, tag="st")
                            nc.scalar.activation(out=st[:], in_=pst[:],
                                                 func=Act.Copy,
                                                 scale=dinvn_t[:, w:w + 1])
                            if w < WA:
                                nc.scalar.dma_start(
                                    out=slA_t[w * P:(w + 1) * P, :], in_=st[:])
                            else:
                                nc.scalar.dma_start(
                                    out=slB_t[(w - WA) * P:(w - WA + 1) * P, :],
                                    in_=st[:])
                        else:
                            # k == 3, last layer: log_softmax epilogue
                            pst = psT.tile([P, P], fp, tag="pst32")
                            nc.tensor.transpose(out=pst[:, :CLS],
                                                in_=acc[:CLS, wb],
                                                identity=ident_t[:CLS, :CLS])
                            nm = stp.tile([P, 1], fp, tag="nm")
                            nc.vector.tensor_reduce(
                                out=nm[:], in_=pst[:, :CLS], op=Alu.max,
                                axis=mybir.AxisListType.X, negate=True)
                            ex = stp.tile([P, CLS], fp, tag="ex")
                            ssum = stp.tile([P, 1], fp, tag="ssum")
                            nc.scalar.activation(out=ex[:], in_=pst[:, :CLS],
                                                 func=Act.Exp, bias=nm[:, 0:1],
                                                 accum_out=ssum[:, 0:1])
                            lse = stp.tile([P, 1], fp, tag="lse")
                            nc.scalar.activation(out=lse[:], in_=ssum[:],
                                                 func=Act.Ln)
                            res = stp.tile([P, CLS], fp, tag="res")
                            nc.vector.tensor_scalar(
                                out=res[:], in0=pst[:, :CLS],
                                scalar1=nm[:, 0:1], scalar2=lse[:, 0:1],
                                op0=Alu.add, op1=Alu.subtract)
                            nc.scalar.dma_start(out=out_d[w * P:(w + 1) * P, :],
                                                in_=res[:])
                    # fire the A-half collective once windows 0..WA-1 done
                    if g == agA_g and (mk_slice or not last):
                        tabA_new = tabAp.tile([RA, F], bf, tag="tabA",
                                              addr_space="Shared")
                        nc.gpsimd.collective_compute(
                            "AllGather", Alu.bypass,
                            replica_groups=[list(range(CORES))],
                            ins=[slA_t[:, :].opt()],
                            outs=[tabA_new[:, :].opt()])
                        hctx2 = tabA_new
                    elif g == agA_g:
                        hctx2 = None
                if mk_slice or not last:
                    tabB_new = tabBp.tile([RB, F], bf, tag="tabB",
                                          addr_space="Shared")
                    nc.gpsimd.collective_compute(
                        "AllGather", Alu.bypass,
                        replica_groups=[list(range(CORES))],
                        ins=[slB_t[:, :].opt()],
                        outs=[tabB_new[:, :].opt()])
                    return tx_new, hctx2, tabB_new
                return tx_new, None, None

            tabA_cur = xA_d[0:RA, :]
            tabB_cur = xB_d[0:RB, :]
            for l, (Wt, b_t, fo) in enumerate(
                    ((w0_t, b0_t, HID), (w1_t, b1_t, HID), (w2_t, b2_t, CLS))):
                if l * 10 >= stage:
                    break
                last = l == 2
                acc = accp.tile([P, SL], fp, tag="acc")
                # ---- k=0 term ----
                for w in range(NW):
                    wb = slice(w * P, (w + 1) * P)
                    psw = psW.tile([P, P], fp, tag="psw")
                    nc.tensor.matmul(out=psw[:fo, :], lhsT=Wt[:, 0, :fo],
                                     rhs=tx0[:, wb], start=True, stop=True)
                    nc.vector.tensor_scalar(
                        out=acc[:fo, wb], in0=psw[:fo, :],
                        scalar1=b_t[:fo, 0:1], scalar2=None, op0=Alu.add)
                # ---- hops ----
                hT = (None if last
                      else txp.tile([P, SL], bf, tag="tx", name="hT"))
                tx1 = tx2 = tx3 = None
                for k in (1, 2, 3):
                    if stage < l * 10 + k + 1:
                        break
                    mk_slice = k < 3
                    slA_t = (slAp.tile([NA, F], bf, tag="slA", name="slA_t")
                             if (mk_slice or not last) else None)
                    slB_t = (slBp.tile([NB, F], bf, tag="slB", name="slB_t")
                             if (mk_slice or not last) else None)
                    prev2 = None if k == 1 else (tx0 if k == 2 else tx1)
                    txn, tA, tB = spmm(tabA_cur, tabB_cur, prev2, Wt, fo, acc,
                                       k, (last, hT, slA_t, slB_t))
                    if k == 1:
                        tx1 = txn
                    elif k == 2:
                        tx2 = txn
                    else:
                        tx3 = txn
                    if tA is not None:
                        tabA_cur = tA[0:RA, :]
                        tabB_cur = tB[0:RB, :]
                if not last:
                    tx0 = hT

    nc.compile()
    return nc


_CACHE = {}


def _get_nc(sched):
    key = (tuple(np.asarray(sched["C"]).flatten().tolist()),
           sched["NCH"], sched["nchc"], sched["wcut"],
           sched.get("STAGE", 99))
    if key not in _CACHE:
        _CACHE[key] = _build(sched)
    return _CACHE[key]


def _run(x, edge_src, edge_dst, W0, b0, W1, b1, W2, b2, cfg=None,
         trace=False, trace_cores=None):
    from concourse import bass_utils
    import ml_dtypes
    bf16 = ml_dtypes.bfloat16

    n = x.shape[0]
    pre = _preprocess(edge_src, edge_dst, n)
    if cfg and "STAGE" in cfg:
        pre["STAGE"] = cfg["STAGE"]
    perm = pre["perm"]

    x = np.asarray(x, np.float32)
    x_pad = np.zeros((NPAD, F), np.float32)
    x_pad[perm] = x
    xs = x_pad * pre["dinv_new"][:, None]          # dinv-scaled rows
    xs_c = xs.reshape(CORES, NW, P, F)
    xA = np.ascontiguousarray(
        xs_c[:, :WA].reshape(CORES * NA, F)).astype(bf16)
    xB = np.ascontiguousarray(
        xs_c[:, WA:].reshape(CORES * NB, F)).astype(bf16)

    w0t = np.ascontiguousarray(
        np.transpose(np.asarray(W0, np.float32), (1, 0, 2))).astype(bf16)
    w1t = np.ascontiguousarray(
        np.transpose(np.asarray(W1, np.float32), (1, 0, 2))).astype(bf16)
    w2t = np.ascontiguousarray(
        np.transpose(np.asarray(W2, np.float32), (1, 0, 2))).astype(bf16)
    iota = np.broadcast_to(np.arange(P, dtype=np.float32), (P, P))
    ident = np.eye(P, dtype=np.float32)

    in_maps = []
    for c in range(CORES):
        rows = slice(c * SL, (c + 1) * SL)
        in_maps.append(dict(
            xT=np.ascontiguousarray(x_pad[rows].T).astype(bf16),
            xA=xA, xB=xB,
            idxs=pre["idxs"][c],
            mcache=pre["mcache"][c],
            dinvd2=pre["dinvd2"][c],
            dinvn=pre["dinvn"][c],
            w0t=w0t, w1t=w1t, w2t=w2t,
            b0=np.asarray(b0, np.float32).reshape(HID, 1),
            b1=np.asarray(b1, np.float32).reshape(HID, 1),
            b2=np.asarray(b2, np.float32).reshape(CLS, 1),
            identb=ident.astype(bf16), ident=ident,
        ))

    nc = _get_nc(pre)
    kw = {}
    if trace:
        kw = dict(trace=True,
                  trace_cores=trace_cores if trace_cores is not None else [0])
    res = bass_utils.run_bass_kernel_spmd(nc, in_maps,
                                          core_ids=list(range(CORES)), **kw)

    full = np.concatenate([res.results[c]["out"] for c in range(CORES)],
                          axis=0)
    out = full[perm]
    return out.astype(np.float32), res


def kernel(x, edge_src, edge_dst, W0, b0, W1, b1, W2, b2):
    out, _ = _run(x, edge_src, edge_dst, W0, b0, W1, b1, W2, b2)
    return out
